# revision 1
# baseline (speedup 1.0000x reference)
"""CrystalGraphConvNet forward pass as a distributed Bass/Tile kernel on 8 TRN2
NeuronCores.

Strategy (graph/data parallel, per sharding hint):
  - Atoms sharded contiguously across 8 cores (7500 each, padded to 7552).
  - Per conv layer, each core computes Y_self = x @ Wf[:F], Y_nbr = x @ Wf[F:2F]
    for its atom shard; Y_nbr shards are AllGathered into a replicated fp16
    table, and neighbor contributions are fetched with dma_gather (transposed,
    channel-major).  int16 gather indices can't span 60k rows, so each edge
    is gathered twice (lo/hi half of the table) with misses redirected to a
    zero row, and the two results are summed.
  - nbr_fea is host-transposed to channel-major fp16; its projection plus the
    self term are accumulated in PSUM via TensorE (self term uses a constant
    block-band one-hot so the per-atom row broadcasts over its 12 edges).
  - Training-mode batchnorm needs global stats, so gated values are staged to
    DRAM scratch in fp16 while per-channel sum/sumsq accumulate; a tiny
    AllReduce yields the affine, which is fused into the sigmoid/softplus
    activations on the second pass.  Sigmoid and softplus live in different
    ACT table sets, so the second pass is split into a batched sigmoid pass
    (results parked in SBUF) and a batched softplus+product+neighbor-sum pass.
  - Crystal mean-pooling is a per-tile one-hot matmul into a 512-crystal
    window, scattered by int32 indirect DMA into a global crystal array and
    AllReduced; the tiny MLP head runs redundantly on every core.
"""

import math
import os
import numpy as np

import concourse.bass as bass
import concourse.bacc as bacc
import concourse.tile as tile
from concourse import mybir
from contextlib import ExitStack

F16 = mybir.dt.float16
F32 = mybir.dt.float32
I16 = mybir.dt.int16
I32 = mybir.dt.int32

SPLIT = 32768  # int16 gather index limit


def make_cfg(N=60000, M=12, F0=92, FB=41, F=64, H=128, NC=2000, NCONV=3,
             EPS=1e-5, NCORES=8, TA=128):
    c = dict(N=N, M=M, F0=F0, FB=FB, F=F, H=H, NC=NC, NCONV=NCONV, EPS=EPS,
             NCORES=NCORES, TA=TA)
    assert N % NCORES == 0
    c["A_shard"] = N // NCORES
    c["ntile"] = (c["A_shard"] + TA - 1) // TA
    c["A_pad"] = c["ntile"] * TA
    c["E_tile"] = TA * M
    c["E_loc"] = c["ntile"] * c["E_tile"]
    c["R"] = NCORES * c["A_pad"] + 2            # table rows (zero row at 0 and R-1)
    c["SPLIT"] = SPLIT
    c["HB"] = min(c["SPLIT"], c["R"] - 2)       # hi-gather base row
    c["NCB"] = 512 * ((NC + 511) // 512) + 512  # crystal bounce rows
    c["G"] = 2 * F                              # gated channels
    return c


CFG = make_cfg()


# --------------------------------------------------------------------------
# program builder
# --------------------------------------------------------------------------

def build_program(c, debug=False, dbg_dump=False, stop=None):
    # stop: optional (layer, stage) tuple for bisection; stages within a layer:
    # 0=Y, 1=AG, 2=A, 3=AR1, 4=B1, 5=B2, 6=AR2, 7=C; (L,0)=pool, (L,1)=head
    nc = bacc.Bacc("TRN2", target_bir_lowering=False, debug=debug,
                   num_devices=c["NCORES"])

    N, M, F0, FB, F, H, NC, L = (c["N"], c["M"], c["F0"], c["FB"], c["F"],
                                 c["H"], c["NC"], c["NCONV"])
    G, TA, NT, AP_, ET, EL = (c["G"], c["TA"], c["ntile"], c["A_pad"],
                              c["E_tile"], c["E_loc"])
    R, HB, NCB, EPS = c["R"], c["HB"], c["NCB"], c["EPS"]
    NPAIR = (NT + 1) // 2  # B-phase processes tiles in pairs (last may be single)

    # ---------------- inputs ----------------
    afT = nc.dram_tensor("afT", [F0, AP_], F16, kind="ExternalInput")
    idxc = nc.dram_tensor("idxc", [NT, 128, 2 * (ET // 16)], I16,
                          kind="ExternalInput")
    nbrT = nc.dram_tensor("nbrT", [FB, EL], F16, kind="ExternalInput")
    oh_self = nc.dram_tensor("oh_self", [128, ET], F16, kind="ExternalInput")
    pone = nc.dram_tensor("pone", [NT, 4, 128, TA], F16, kind="ExternalInput")
    scidx = nc.dram_tensor("scidx", [128, 4], I32, kind="ExternalInput")
    invcnt = nc.dram_tensor("invcnt", [NCB, 1], F32, kind="ExternalInput")
    w_emb = nc.dram_tensor("w_emb", [F0, F], F16, kind="ExternalInput")
    b_emb = nc.dram_tensor("b_emb", [F, 1], F32, kind="ExternalInput")
    w_self = nc.dram_tensor("w_self", [L, F, G], F16, kind="ExternalInput")
    w_nbr = nc.dram_tensor("w_nbr", [L, F, G], F16, kind="ExternalInput")
    w_b = nc.dram_tensor("w_b", [L, FB, G], F16, kind="ExternalInput")
    g1 = nc.dram_tensor("g1", [L, G, 1], F32, kind="ExternalInput")
    be1 = nc.dram_tensor("be1", [L, G, 1], F32, kind="ExternalInput")
    g2 = nc.dram_tensor("g2", [L, F, 1], F32, kind="ExternalInput")
    be2 = nc.dram_tensor("be2", [L, F, 1], F32, kind="ExternalInput")
    w_fc = nc.dram_tensor("w_fc", [F, H], F16, kind="ExternalInput")
    b_fc = nc.dram_tensor("b_fc", [H, 1], F32, kind="ExternalInput")
    w_out = nc.dram_tensor("w_out", [H, 1], F16, kind="ExternalInput")
    b_out = nc.dram_tensor("b_out", [1, 1], F32, kind="ExternalInput")
    ident = nc.dram_tensor("ident", [128, 128], F32, kind="ExternalInput")

    out_t = nc.dram_tensor("out", [1, NCB], F32, kind="ExternalOutput")
    if dbg_dump:
        L_, F_, G_, AP2, EL_ = c["NCONV"], c["F"], c["G"], c["A_pad"], c["E_loc"]
        dbgx0 = nc.dram_tensor("dbgx0", [F_, AP2], F32, kind="ExternalOutput")
        dbgx = nc.dram_tensor("dbgx", [L_, F_, AP2], F32, kind="ExternalOutput")
        dbgsum = nc.dram_tensor("dbgsum", [L_, F_, AP2], F32, kind="ExternalOutput")
        dbgst1 = nc.dram_tensor("dbgst1", [L_, G_, 2], F32, kind="ExternalOutput")
        dbggat = nc.dram_tensor("dbggat", [L_, F_, EL_], F32, kind="ExternalOutput")
        dbgpool = nc.dram_tensor("dbgpool", [NCB, F_], F32, kind="ExternalOutput")

    # ---------------- internal DRAM ----------------
    yb = nc.dram_tensor("yb", [AP_, G], F16)                       # AG input bounce
    tbl = nc.dram_tensor("tbl", [R, G], F16, addr_space="Shared")  # AG output table
    scrF = nc.dram_tensor("scrF", [F, EL], F16)
    scrC = nc.dram_tensor("scrC", [F, EL], F16)
    ar1_in = nc.dram_tensor("ar1_in", [G, 2], F32)
    ar1_out = nc.dram_tensor("ar1_out", [G, 2], F32, addr_space="Shared")
    ar2_in = nc.dram_tensor("ar2_in", [F, 2], F32)
    ar2_out = nc.dram_tensor("ar2_out", [F, 2], F32, addr_space="Shared")
    pool_in = nc.dram_tensor("pool_in", [NCB, F], F32)
    pool_out = nc.dram_tensor("pool_out", [NCB, F], F32, addr_space="Shared")

    rg = [list(range(c["NCORES"]))]
    AF = mybir.ActivationFunctionType
    OP = mybir.AluOpType
    if stop is None:
        stop = (L, 9)

    def live(key):
        return key <= stop

    with tile.TileContext(nc) as tc, ExitStack() as top:
        # persistent SBUF state
        x_cm = nc.alloc_sbuf_tensor("x_cm", [F, AP_], F32)
        summed = nc.alloc_sbuf_tensor("summed", [F, AP_], F16)
        ysr = nc.alloc_sbuf_tensor("ysr", [128, NT, G], F16)      # Y_self row-major
        sig_all = nc.alloc_sbuf_tensor("sig_all", [128, NPAIR * ET], F16)

        const = top.enter_context(tc.tile_pool(name="const", bufs=1))
        stats = top.enter_context(tc.tile_pool(name="stats", bufs=1))

        # constants resident all kernel
        ohs_t = const.tile([128, ET], F16)
        nc.sync.dma_start(out=ohs_t[:], in_=oh_self[:, :])
        wemb_t = const.tile([F0, F], F16)
        nc.sync.dma_start(out=wemb_t[:], in_=w_emb[:, :])
        bemb_t = const.tile([F, 1], F32)
        nc.sync.dma_start(out=bemb_t[:], in_=b_emb[:, :])
        id_t = const.tile([128, 128], F32)
        nc.sync.dma_start(out=id_t[:], in_=ident[:, :])
        eps_t = const.tile([128, 1], F32)
        nc.vector.memset(eps_t[:], EPS)
        zrow = const.tile([1, G], F16)
        nc.vector.memset(zrow[:], 0.0)
        zt128 = const.tile([128, F], F32)
        nc.vector.memset(zt128[:], 0.0)

        wS = []
        wN = []
        wB = []
        for l in range(L):
            t1 = const.tile([F, G], F16, tag=f"wS{l}")
            nc.sync.dma_start(out=t1[:], in_=w_self[l, :, :])
            t2 = const.tile([F, G], F16, tag=f"wN{l}")
            nc.sync.dma_start(out=t2[:], in_=w_nbr[l, :, :])
            t3 = const.tile([FB, G], F16, tag=f"wB{l}")
            nc.sync.dma_start(out=t3[:], in_=w_b[l, :, :])
            wS.append(t1)
            wN.append(t2)
            wB.append(t3)

        # stats buffers
        st1_s = stats.tile([G, NT], F32, tag="st1s")
        st1_q = stats.tile([G, NT], F32, tag="st1q")
        st2_s = stats.tile([F, 2 * NPAIR], F32, tag="st2s")
        st2_q = stats.tile([F, 2 * NPAIR], F32, tag="st2q")

        # zero table guard rows + summed pads
        nc.sync.dma_start(out=tbl[0:1, :], in_=zrow[:])
        nc.sync.dma_start(out=tbl[R - 1:R, :], in_=zrow[:])
        nc.vector.memset(summed[:, :], 0.0)

        # ---------------- embedding: x = atom_fea @ W_emb + b_emb ----------
        with tc.tile_pool(name="emb", bufs=3) as embp, \
             tc.tile_pool(name="embps", bufs=2, space="PSUM") as embps:
            CH = 512
            for j in range(0, AP_, CH):
                w = min(CH, AP_ - j)
                rhs = embp.tile([F0, CH], F16, tag="embr")
                nc.sync.dma_start(out=rhs[:, :w], in_=afT[:, j:j + w])
                ps = embps.tile([F, CH], F32, tag="embp")
                nc.tensor.matmul(ps[:, :w], lhsT=wemb_t[:], rhs=rhs[:, :w],
                                 start=True, stop=True)
                nc.scalar.activation(out=x_cm[:, j:j + w], in_=ps[:, :w],
                                     func=AF.Identity, bias=bemb_t[:], scale=1.0)
        if dbg_dump:
            nc.sync.dma_start(out=dbgx0[:, :], in_=x_cm[:, :])

        # ---------------- conv layers ----------------
        for l in range(L):
            if not live((l, 0)):
                break
            # ---- phase Y: Y_self (SBUF) / Y_nbr (-> bounce -> AllGather) ----
            with tc.tile_pool(name="yph", bufs=3) as yp, \
                 tc.tile_pool(name="yps", bufs=2, space="PSUM") as yps:
                lastreal = c["A_shard"] - (NT - 1) * TA
                for t in range(NT):
                    xa = yp.tile([F, TA], F16, tag="xa")
                    nc.scalar.activation(out=xa[:], in_=x_cm[:, t * TA:(t + 1) * TA],
                                         func=AF.Copy)
                    psS = yps.tile([TA, G], F32, tag="psS")
                    nc.tensor.matmul(psS[:], lhsT=xa[:], rhs=wS[l][:],
                                     start=True, stop=True)
                    # pad atoms of the last tile must contribute exactly zero
                    # through the self one-hot matmul
                    nreal = TA if t < NT - 1 else lastreal
                    if nreal < TA:
                        nc.vector.memset(ysr[:, t, :], 0.0)
                    nc.scalar.activation(out=ysr[0:nreal, t, :],
                                         in_=psS[0:nreal, :], func=AF.Copy)
                    psN = yps.tile([TA, G], F32, tag="psN")
                    nc.tensor.matmul(psN[:], lhsT=xa[:], rhs=wN[l][:],
                                     start=True, stop=True)
                    yn = yp.tile([TA, G], F16, tag="yn")
                    nc.scalar.activation(out=yn[:], in_=psN[:], func=AF.Copy)
                    nc.sync.dma_start(out=yb[t * TA:(t + 1) * TA, :], in_=yn[:])

            if not live((l, 1)):
                break
            tc.strict_bb_all_engine_barrier()
            nc.gpsimd.collective_compute(
                "AllGather", OP.bypass, replica_groups=rg,
                ins=[yb[:, :]], outs=[tbl[1:R - 1, :]])
            tc.strict_bb_all_engine_barrier()

            if not live((l, 2)):
                break
            # ---- pass A: edges -> gated scratch + stats1 ----
            with tc.tile_pool(name="pa", bufs=3) as pa, \
                 tc.tile_pool(name="paps", bufs=2, space="PSUM") as paps:
                W16 = ET // 16
                for t in range(NT):
                    idx_t = pa.tile([128, 2 * W16], I16, tag="idx")
                    nc.sync.dma_start(out=idx_t[:], in_=idxc[t, :, :])
                    nbt = pa.tile([FB, ET], F16, tag="nbt")
                    nc.sync.dma_start(out=nbt[:], in_=nbrT[:, t * ET:(t + 1) * ET])

                    glo = pa.tile([128, 1, ET], F16, tag="glo")
                    nc.gpsimd.dma_gather(glo[:], tbl[0:HB, :], idx_t[:, 0:W16],
                                         ET, ET, G, transpose=True,
                                         single_packet=False)
                    ghi = pa.tile([128, 1, ET], F16, tag="ghi")
                    nc.gpsimd.dma_gather(ghi[:], tbl[HB:R, :], idx_t[:, W16:2 * W16],
                                         ET, ET, G, transpose=True,
                                         single_packet=False)

                    ps = paps.tile([G, ET], F32, tag="aps")
                    for s in range(ET // 512):
                        sl = slice(s * 512, (s + 1) * 512)
                        nc.tensor.matmul(ps[:, sl], lhsT=wB[l][:], rhs=nbt[:, sl],
                                         start=True, stop=False)
                        nc.tensor.matmul(ps[:, sl], lhsT=ysr[:, t, :],
                                         rhs=ohs_t[:, sl], start=False, stop=True)

                    nc.vector.tensor_tensor(out=glo[:, 0, :], in0=glo[:, 0, :],
                                            in1=ghi[:, 0, :], op=OP.add)
                    gat = pa.tile([128, ET], F16, tag="gat")
                    nc.vector.scalar_tensor_tensor(
                        out=gat[:], in0=glo[:, 0, :], scalar=0.0, in1=ps[:],
                        op0=OP.add, op1=OP.add,
                        accum_out=st1_s[:, t:t + 1])
                    nc.scalar.activation(out=ghi[:, 0, :], in_=gat[:],
                                         func=AF.Square,
                                         accum_out=st1_q[:, t:t + 1])
                    nc.sync.dma_start(out=scrF[:, t * ET:(t + 1) * ET],
                                      in_=gat[0:F, :])
                    nc.sync.dma_start(out=scrC[:, t * ET:(t + 1) * ET],
                                      in_=gat[F:G, :])

            if not live((l, 3)):
                break
            # ---- stats1 reduce + AllReduce + affine ----
            with tc.tile_pool(name="s1", bufs=1) as s1p:
                pack1 = s1p.tile([G, 2], F32, tag="pack1")
                nc.vector.tensor_reduce(out=pack1[:, 0:1], in_=st1_s[:],
                                        axis=mybir.AxisListType.X, op=OP.add)
                nc.vector.tensor_reduce(out=pack1[:, 1:2], in_=st1_q[:],
                                        axis=mybir.AxisListType.X, op=OP.add)
                nc.sync.dma_start(out=ar1_in[:, :], in_=pack1[:])
                tc.strict_bb_all_engine_barrier()
                nc.gpsimd.collective_compute(
                    "AllReduce", OP.add, replica_groups=rg,
                    ins=[ar1_in[:, :]], outs=[ar1_out[:, :]])
                tc.strict_bb_all_engine_barrier()

                red1 = s1p.tile([G, 2], F32, tag="red1")
                nc.sync.dma_start(out=red1[:], in_=ar1_out[:, :])
                if dbg_dump:
                    nc.sync.dma_start(out=dbgst1[l, :, :], in_=red1[:])
                    nc.gpsimd.dma_start(out=dbggat[l, :, :], in_=scrF[:, :])
                g1_t = s1p.tile([G, 1], F32, tag="g1t")
                nc.sync.dma_start(out=g1_t[:], in_=g1[l, :, :])
                be1_t = s1p.tile([G, 1], F32, tag="be1t")
                nc.sync.dma_start(out=be1_t[:], in_=be1[l, :, :])

                # rsqrt(var+eps) = exp(-0.5*ln(var+eps)) + one Newton step
                # (no sqrt table needed; Ln/Exp share one ACT table set)
                invE = 1.0 / float(N * M)
                mmt = s1p.tile([G, 8], F32, tag="mmt")
                mcol = mmt[:, 0:1]
                nc.vector.tensor_scalar(out=mcol, in0=red1[:, 0:1], scalar1=invE,
                                        scalar2=None, op0=OP.mult)
                ex2 = mmt[:, 1:2]
                nc.vector.tensor_scalar(out=ex2, in0=red1[:, 1:2], scalar1=invE,
                                        scalar2=None, op0=OP.mult)
                msq = mmt[:, 2:3]
                nc.vector.tensor_tensor(out=msq, in0=mcol, in1=mcol, op=OP.mult)
                var = mmt[:, 3:4]
                nc.vector.tensor_tensor(out=var, in0=ex2, in1=msq, op=OP.subtract)
                lv = mmt[:, 4:5]
                nc.scalar.activation(out=lv, in_=var, func=AF.Ln,
                                     bias=eps_t[0:G, :], scale=1.0)
                r0 = mmt[:, 5:6]
                nc.scalar.activation(out=r0, in_=lv, func=AF.Exp, scale=-0.5)
                # one Newton step: r1 = r0*(1.5 - 0.5*(var+eps)*r0^2)
                vpe = mmt[:, 6:7]
                nc.vector.tensor_scalar(out=vpe, in0=var, scalar1=eps_t[0:G, :],
                                        scalar2=0.5, op0=OP.add, op1=OP.mult)
                r0q = mmt[:, 7:8]
                nc.vector.tensor_tensor(out=r0q, in0=r0, in1=r0, op=OP.mult)
                nc.vector.tensor_tensor(out=r0q, in0=r0q, in1=vpe, op=OP.mult)
                nc.vector.tensor_scalar(out=r0q, in0=r0q, scalar1=-1.0,
                                        scalar2=1.5, op0=OP.mult, op1=OP.add)
                r1 = mmt[:, 6:7]
                nc.vector.tensor_tensor(out=r1, in0=r0, in1=r0q, op=OP.mult)

                s1c_ = s1p.tile([G, 1], F32, tag="s1c")
                nc.vector.tensor_tensor(out=s1c_[:], in0=g1_t[:], in1=r1,
                                        op=OP.mult)
                t1c_ = s1p.tile([G, 1], F32, tag="t1c")
                nc.vector.tensor_tensor(out=t1c_[:], in0=mcol, in1=s1c_[:],
                                        op=OP.mult)
                nc.vector.scalar_tensor_tensor(out=t1c_[:], in0=t1c_[:],
                                               scalar=-1.0, in1=be1_t[:],
                                               op0=OP.mult, op1=OP.add)
                # replicated (packed-pair) scale/bias
                sF = s1p.tile([128, 1], F32, tag="sF")
                tF = s1p.tile([128, 1], F32, tag="tF")
                sC = s1p.tile([128, 1], F32, tag="sC")
                tC = s1p.tile([128, 1], F32, tag="tC")
                for half in range(2):
                    hp = slice(half * F, half * F + F)
                    nc.sync.dma_start(out=sF[hp, :], in_=s1c_[0:F, :])
                    nc.sync.dma_start(out=tF[hp, :], in_=t1c_[0:F, :])
                    nc.sync.dma_start(out=sC[hp, :], in_=s1c_[F:G, :])
                    nc.sync.dma_start(out=tC[hp, :], in_=t1c_[F:G, :])

                if not live((l, 4)):
                    break
                # ---- B1: batched sigmoid over filter halves ----
                tc.strict_bb_all_engine_barrier()
                with tc.tile_pool(name="b1", bufs=3) as b1p:
                    for p in range(NPAIR):
                        t0, t1_ = 2 * p, min(2 * p + 1, NT - 1)
                        single = (2 * p + 1 > NT - 1)
                        fin = b1p.tile([128, ET], F16, tag="fin")
                        nc.sync.dma_start(out=fin[0:F, :],
                                          in_=scrF[:, t0 * ET:(t0 + 1) * ET])
                        if not single:
                            nc.sync.dma_start(out=fin[F:G, :],
                                              in_=scrF[:, t1_ * ET:(t1_ + 1) * ET])
                        pp = 128 if not single else F
                        nc.scalar.activation(
                            out=sig_all[0:pp, p * ET:(p + 1) * ET],
                            in_=fin[0:pp, :], func=AF.Sigmoid,
                            bias=tF[0:pp, :], scale=sF[0:pp, :])

                tc.no_sync_barrier()

                if not live((l, 5)):
                    break
                # ---- B2: softplus, product, neighbor-sum, stats2 ----
                with tc.tile_pool(name="b2", bufs=3) as b2p:
                    for p in range(NPAIR):
                        t0, t1_ = 2 * p, min(2 * p + 1, NT - 1)
                        single = (2 * p + 1 > NT - 1)
                        pp = 128 if not single else F
                        cin = b2p.tile([128, ET], F16, tag="cin")
                        nc.sync.dma_start(out=cin[0:F, :],
                                          in_=scrC[:, t0 * ET:(t0 + 1) * ET])
                        if not single:
                            nc.sync.dma_start(out=cin[F:G, :],
                                              in_=scrC[:, t1_ * ET:(t1_ + 1) * ET])
                        # softplus(z) = ln(1 + exp(z)); no softplus ACT table
                        # exists in this toolchain
                        expv = b2p.tile([128, ET], F16, tag="expv")
                        nc.scalar.activation(out=expv[0:pp, :], in_=cin[0:pp, :],
                                             func=AF.Exp,
                                             bias=tC[0:pp, :], scale=sC[0:pp, :])
                        spv = b2p.tile([128, ET], F16, tag="spv")
                        nc.scalar.activation(out=spv[0:pp, :], in_=expv[0:pp, :],
                                             func=AF.Ln, bias=1.0, scale=1.0)
                        prod = b2p.tile([128, ET], F16, tag="prod")
                        nc.vector.tensor_tensor(
                            out=prod[0:pp, :],
                            in0=sig_all[0:pp, p * ET:(p + 1) * ET],
                            in1=spv[0:pp, :], op=OP.mult)
                        halves = 1 if single else 2
                        for hh in range(halves):
                            tt = t0 if hh == 0 else t1_
                            hs = slice(hh * F, hh * F + F)
                            red = b2p.tile([F, TA], F32, tag="red")
                            nc.vector.tensor_reduce(
                                out=red[:],
                                in_=prod[hs, :].rearrange("p (a m) -> p a m", m=M),
                                axis=mybir.AxisListType.X, op=OP.add)
                            nc.scalar.activation(
                                out=summed[:, tt * TA:(tt + 1) * TA],
                                in_=red[:], func=AF.Copy)
                            # stats over real atoms only
                            nreal = min(c["A_shard"] - tt * TA, TA)
                            col = 2 * p + hh
                            nc.vector.tensor_reduce(
                                out=st2_s[:, col:col + 1], in_=red[:, 0:nreal],
                                axis=mybir.AxisListType.X, op=OP.add)
                            sqt = b2p.tile([F, TA], F32, tag="sqt")
                            nc.scalar.activation(
                                out=sqt[:, 0:nreal], in_=red[:, 0:nreal],
                                func=AF.Square,
                                accum_out=st2_q[:, col:col + 1])

                if not live((l, 6)):
                    break
                # ---- stats2 AllReduce + affine2 + pass C ----
                ncols = NT
                pack2 = s1p.tile([F, 2], F32, tag="pack2")
                nc.vector.tensor_reduce(out=pack2[:, 0:1],
                                        in_=st2_s[:, 0:ncols],
                                        axis=mybir.AxisListType.X, op=OP.add)
                nc.vector.tensor_reduce(out=pack2[:, 1:2],
                                        in_=st2_q[:, 0:ncols],
                                        axis=mybir.AxisListType.X, op=OP.add)
                nc.sync.dma_start(out=ar2_in[:, :], in_=pack2[:])
                tc.strict_bb_all_engine_barrier()
                nc.gpsimd.collective_compute(
                    "AllReduce", OP.add, replica_groups=rg,
                    ins=[ar2_in[:, :]], outs=[ar2_out[:, :]])
                tc.strict_bb_all_engine_barrier()

                red2 = s1p.tile([F, 2], F32, tag="red2")
                nc.sync.dma_start(out=red2[:], in_=ar2_out[:, :])
                g2_t = s1p.tile([F, 1], F32, tag="g2t")
                nc.sync.dma_start(out=g2_t[:], in_=g2[l, :, :])
                be2_t = s1p.tile([F, 1], F32, tag="be2t")
                nc.sync.dma_start(out=be2_t[:], in_=be2[l, :, :])

                invN = 1.0 / float(N)
                mt2 = s1p.tile([F, 8], F32, tag="mt2")
                m2c = mt2[:, 0:1]
                nc.vector.tensor_scalar(out=m2c, in0=red2[:, 0:1], scalar1=invN,
                                        scalar2=None, op0=OP.mult)
                e2c = mt2[:, 1:2]
                nc.vector.tensor_scalar(out=e2c, in0=red2[:, 1:2], scalar1=invN,
                                        scalar2=None, op0=OP.mult)
                ms2 = mt2[:, 2:3]
                nc.vector.tensor_tensor(out=ms2, in0=m2c, in1=m2c, op=OP.mult)
                v2 = mt2[:, 3:4]
                nc.vector.tensor_tensor(out=v2, in0=e2c, in1=ms2, op=OP.subtract)
                lv2 = mt2[:, 4:5]
                nc.scalar.activation(out=lv2, in_=v2, func=AF.Ln,
                                     bias=eps_t[0:F, :], scale=1.0)
                r02 = mt2[:, 5:6]
                nc.scalar.activation(out=r02, in_=lv2, func=AF.Exp, scale=-0.5)
                vpe2 = mt2[:, 6:7]
                nc.vector.tensor_scalar(out=vpe2, in0=v2, scalar1=eps_t[0:F, :],
                                        scalar2=0.5, op0=OP.add, op1=OP.mult)
                r0q2 = mt2[:, 7:8]
                nc.vector.tensor_tensor(out=r0q2, in0=r02, in1=r02, op=OP.mult)
                nc.vector.tensor_tensor(out=r0q2, in0=r0q2, in1=vpe2, op=OP.mult)
                nc.vector.tensor_scalar(out=r0q2, in0=r0q2, scalar1=-1.0,
                                        scalar2=1.5, op0=OP.mult, op1=OP.add)
                r12 = mt2[:, 6:7]
                nc.vector.tensor_tensor(out=r12, in0=r02, in1=r0q2, op=OP.mult)
                s2c = s1p.tile([F, 1], F32, tag="s2c")
                nc.vector.tensor_tensor(out=s2c[:], in0=g2_t[:], in1=r12,
                                        op=OP.mult)
                t2c = s1p.tile([F, 1], F32, tag="t2c")
                nc.vector.tensor_tensor(out=t2c[:], in0=m2c, in1=s2c[:],
                                        op=OP.mult)
                nc.vector.scalar_tensor_tensor(out=t2c[:], in0=t2c[:],
                                               scalar=-1.0, in1=be2_t[:],
                                               op0=OP.mult, op1=OP.add)

                if not live((l, 7)):
                    break
                # pass C: x = softplus(x + s2*summed + t2) via ln(1+exp)
                with tc.tile_pool(name="pc", bufs=2) as pcp:
                    CW = 2048
                    for j in range(0, AP_, CW):
                        w = min(CW, AP_ - j)
                        pre = pcp.tile([F, CW], F32, tag="pre")
                        nc.vector.scalar_tensor_tensor(
                            out=pre[:, :w], in0=summed[:, j:j + w],
                            scalar=s2c[:, 0:1], in1=x_cm[:, j:j + w],
                            op0=OP.mult, op1=OP.add)
                        pex = pcp.tile([F, CW], F32, tag="pex")
                        nc.scalar.activation(out=pex[:, :w], in_=pre[:, :w],
                                             func=AF.Exp, bias=t2c[:], scale=1.0)
                        nc.scalar.activation(out=x_cm[:, j:j + w], in_=pex[:, :w],
                                             func=AF.Ln, bias=1.0, scale=1.0)
                if dbg_dump:
                    nc.gpsimd.dma_start(out=dbgsum[l, :, :], in_=summed[:, :])
                    nc.sync.dma_start(out=dbgx[l, :, :], in_=x_cm[:, :])

        for _ in range(1 if live((L, 0)) else 0):
            # ---------------- crystal pooling ----------------
            with tc.tile_pool(name="pool", bufs=4) as pp_, \
                 tc.tile_pool(name="poolacc", bufs=1, space="PSUM") as pacc, \
                 tc.tile_pool(name="poolps", bufs=2, space="PSUM") as pps:
                acc = []
                for ci in range(4):
                    acc_t = pacc.tile([128, F], F32, tag=f"pacc{ci}")
                    acc.append(acc_t)
                for t in range(NT):
                    xps = pps.tile([TA, F], F32, tag="xps")
                    nc.tensor.transpose(out=xps[:], in_=x_cm[:, t * TA:(t + 1) * TA],
                                        identity=id_t[0:F, 0:F])
                    xrm = pp_.tile([TA, F], F16, tag="xrm")
                    nc.scalar.activation(out=xrm[:], in_=xps[:], func=AF.Copy)
                    for ci in range(4):
                        oht = pp_.tile([128, TA], F16, tag="oht")
                        nc.sync.dma_start(out=oht[:], in_=pone[t, ci, :, :])
                        nc.tensor.matmul(acc[ci][:], lhsT=oht[:], rhs=xrm[:],
                                         start=(t == 0), stop=(t == NT - 1))
                # zero pool_in, then scatter our window rows
                for r_ in range(NCB // 128):
                    nc.sync.dma_start(out=pool_in[128 * r_:128 * (r_ + 1), :],
                                      in_=zt128[:, 0:F])
                sci = pp_.tile([128, 4], I32, tag="sci")
                nc.sync.dma_start(out=sci[:], in_=scidx[:, :])
                tc.strict_bb_all_engine_barrier()
                accs = []
                for ci in range(4):
                    asb = pp_.tile([128, F], F32, tag=f"asb{ci}")
                    nc.vector.tensor_copy(out=asb[:], in_=acc[ci][:])
                    accs.append(asb)
                for ci in range(4):
                    nc.gpsimd.indirect_dma_start(
                        out=pool_in[:, :],
                        out_offset=bass.IndirectOffsetOnAxis(ap=sci[:, ci:ci + 1], axis=0),
                        in_=accs[ci][:], in_offset=None)
                tc.strict_bb_all_engine_barrier()
                nc.gpsimd.collective_compute(
                    "AllReduce", OP.add, replica_groups=rg,
                    ins=[pool_in[:, :]], outs=[pool_out[:, :]])
                tc.strict_bb_all_engine_barrier()

            if dbg_dump:
                nc.sync.dma_start(out=dbgpool[:, :], in_=pool_out[:, :])


        for _ in range(1 if live((L, 1)) else 0):
            # ---------------- head ----------------
            with tc.tile_pool(name="head", bufs=3) as hp, \
                 tc.tile_pool(name="headps", bufs=2, space="PSUM") as hps:
                crys = nc.alloc_sbuf_tensor("crys", [F, NCB], F16)
                wfc_t = hp.tile([F, H], F16, tag="wfc")
                nc.sync.dma_start(out=wfc_t[:], in_=w_fc[:, :])
                bfc_t = hp.tile([H, 1], F32, tag="bfc")
                nc.sync.dma_start(out=bfc_t[:], in_=b_fc[:, :])
                wout_t = hp.tile([H, 1], F16, tag="wout")
                nc.sync.dma_start(out=wout_t[:], in_=w_out[:, :])
                bout_t = hp.tile([1, 1], F32, tag="bout")
                nc.sync.dma_start(out=bout_t[:], in_=b_out[:, :])

                for r_ in range(NCB // 128):
                    pt = hp.tile([128, F], F32, tag="pt")
                    nc.sync.dma_start(out=pt[:], in_=pool_out[128 * r_:128 * (r_ + 1), :])
                    ic = hp.tile([128, 1], F32, tag="ic")
                    nc.sync.dma_start(out=ic[:], in_=invcnt[128 * r_:128 * (r_ + 1), :])
                    nc.vector.tensor_scalar(out=pt[:], in0=pt[:], scalar1=ic[:, 0:1],
                                            scalar2=None, op0=OP.mult)
                    pex2 = hp.tile([128, F], F32, tag="pex2")
                    nc.scalar.activation(out=pex2[:], in_=pt[:], func=AF.Exp)
                    spt = hp.tile([128, F], F32, tag="spt")
                    nc.scalar.activation(out=spt[:], in_=pex2[:], func=AF.Ln,
                                         bias=1.0, scale=1.0)
                    tps = hps.tile([F, 128], F32, tag="tps")
                    nc.tensor.transpose(out=tps[:], in_=spt[:], identity=id_t[:, :])
                    nc.scalar.activation(out=crys[:, 128 * r_:128 * (r_ + 1)],
                                         in_=tps[:], func=AF.Copy)

                hc = nc.alloc_sbuf_tensor("hc", [H, NCB], F16)
                for j in range(0, NCB, 512):
                    psh = hps.tile([H, 512], F32, tag="psh")
                    nc.tensor.matmul(psh[:], lhsT=wfc_t[:], rhs=crys[:, j:j + 512],
                                     start=True, stop=True)
                    hex_ = hp.tile([H, 512], F32, tag="hex")
                    nc.scalar.activation(out=hex_[:], in_=psh[:],
                                         func=AF.Exp, bias=bfc_t[:], scale=1.0)
                    nc.scalar.activation(out=hc[:, j:j + 512], in_=hex_[:],
                                         func=AF.Ln, bias=1.0, scale=1.0)
                ofin = hp.tile([1, NCB], F32, tag="ofin")
                for j in range(0, NCB, 512):
                    pso = hps.tile([1, 512], F32, tag="pso")
                    nc.tensor.matmul(pso[:], lhsT=wout_t[:], rhs=hc[:, j:j + 512],
                                     start=True, stop=True)
                    nc.scalar.activation(out=ofin[:, j:j + 512], in_=pso[:],
                                         func=AF.Identity, bias=bout_t[:], scale=1.0)
                nc.sync.dma_start(out=out_t[:, :], in_=ofin[:])

    nc.compile()
    return nc


# --------------------------------------------------------------------------
# host-side input preparation
# --------------------------------------------------------------------------

def prepare_inputs(c, atom_fea, nbr_fea, nbr_fea_idx, crystal_atom_idx,
                   W_emb, b_emb, W_full, b_full, g1, be1, g2, be2,
                   W_fc, b_fc, W_out, b_out):
    N, M, F0, FB, F, H, NC, L = (c["N"], c["M"], c["F0"], c["FB"], c["F"],
                                 c["H"], c["NC"], c["NCONV"])
    G, TA, NT, AP_, ET, EL = (c["G"], c["TA"], c["ntile"], c["A_pad"],
                              c["E_tile"], c["E_loc"])
    R, HB, NCB, K = c["R"], c["HB"], c["NCB"], c["NCORES"]
    AS = c["A_shard"]

    atom_fea = np.asarray(atom_fea, np.float32)
    nbr_fea = np.asarray(nbr_fea, np.float32)
    nbr_fea_idx = np.asarray(nbr_fea_idx, np.int64)
    crystal_atom_idx = np.asarray(crystal_atom_idx, np.int64)

    # shared (replicated) tensors
    oh = np.zeros((128, ET), np.float16)
    for j in range(ET):
        oh[j // M, j] = 1.0
    shared = {
        "oh_self": oh,
        "w_emb": np.asarray(W_emb, np.float16),
        "b_emb": np.asarray(b_emb, np.float32).reshape(F, 1),
        "w_self": np.asarray(W_full[:, :F, :], np.float16),
        "w_nbr": np.asarray(W_full[:, F:2 * F, :], np.float16),
        "w_b": np.asarray(W_full[:, 2 * F:, :], np.float16),
        "g1": np.asarray(g1, np.float32).reshape(L, G, 1),
        "be1": np.asarray(be1, np.float32).reshape(L, G, 1),
        "g2": np.asarray(g2, np.float32).reshape(L, F, 1),
        "be2": np.asarray(be2, np.float32).reshape(L, F, 1),
        "w_fc": np.asarray(W_fc, np.float16),
        "b_fc": np.asarray(b_fc, np.float32).reshape(H, 1),
        "w_out": np.asarray(W_out, np.float16),
        "b_out": np.asarray(b_out, np.float32).reshape(1, 1),
        "ident": np.eye(128, dtype=np.float32),
    }
    # crystal counts (global, from index data only)
    cnt = np.bincount(crystal_atom_idx, minlength=NC).astype(np.float32)
    icnt = np.zeros((NCB, 1), np.float32)
    icnt[:NC, 0] = 1.0 / np.maximum(cnt, 1.0)
    shared["invcnt"] = icnt

    # b_full is mathematically irrelevant (cancelled by training-mode BN)

    in_maps = []
    for k in range(K):
        a0 = k * AS
        af = np.zeros((F0, AP_), np.float16)
        af[:, :AS] = atom_fea[a0:a0 + AS].T
        # edge ordering: e = a*M + m within each tile of TA atoms
        gi = np.zeros((NT * TA, M), np.int64)
        gi_raw = nbr_fea_idx[a0:a0 + AS]
        gi[:AS] = gi_raw
        valid = np.zeros((NT * TA, M), bool)
        valid[:AS] = True
        rows = np.where(valid, 1 + (gi // AS) * AP_ + (gi % AS), 0)
        lo = np.where(rows < HB, rows, 0).astype(np.int64)
        hi = np.where(rows >= HB, rows - HB, R - 1 - HB).astype(np.int64)
        W16 = ET // 16
        idxc = np.zeros((NT, 128, 2 * W16), np.int16)
        j = np.arange(ET)
        for t in range(NT):
            fl = lo[t * TA:(t + 1) * TA].reshape(ET)
            fh = hi[t * TA:(t + 1) * TA].reshape(ET)
            wrap_l = np.zeros((16, W16), np.int16)
            wrap_h = np.zeros((16, W16), np.int16)
            wrap_l[j % 16, j // 16] = fl
            wrap_h[j % 16, j // 16] = fh
            idxc[t, :, 0:W16] = np.tile(wrap_l, (8, 1))
            idxc[t, :, W16:] = np.tile(wrap_h, (8, 1))
        nb = np.zeros((FB, EL), np.float16)
        nb_l = nbr_fea[a0:a0 + AS].reshape(AS * M, FB)
        src = np.zeros((NT * TA * M, FB), np.float32)
        src[:AS * M] = nb_l
        # src is already in (a, m) order; tiles are contiguous runs of ET
        nb[:, :] = src.T.astype(np.float16)

        cry = np.zeros(NT * TA, np.int64)
        cry[:AS] = crystal_atom_idx[a0:a0 + AS]
        cb = int(crystal_atom_idx[a0:a0 + AS].min())
        cmax = int(crystal_atom_idx[a0:a0 + AS].max())
        assert cmax - cb < 512, f"crystal window too wide: {cmax - cb}"
        # lhsT layout: [atom (partition=K), crystal-window col (free=M)]
        pone = np.zeros((NT, 4, TA, 128), np.float16)
        for t in range(NT):
            for a in range(TA):
                ga = t * TA + a
                if ga >= AS:
                    continue
                col = int(cry[ga]) - cb
                pone[t, col // 128, a, col % 128] = 1.0
        scidx = (cb + np.arange(512, dtype=np.int32)).reshape(4, 128).T.copy()
        assert cb + 512 <= NCB

        in_maps.append(dict(shared,
                            afT=af, idxc=idxc, nbrT=nb, pone=pone,
                            scidx=scidx))
    return in_maps


# --------------------------------------------------------------------------
# public entry point
# --------------------------------------------------------------------------

_PROG_CACHE = {}


def _get_program(c):
    key = tuple(sorted((k, v) for k, v in c.items()))
    if key not in _PROG_CACHE:
        _PROG_CACHE[key] = build_program(c)
    return _PROG_CACHE[key]


def kernel(atom_fea, nbr_fea, nbr_fea_idx, crystal_atom_idx, W_emb, b_emb,
           W_full, b_full, g1, be1, g2, be2, W_fc, b_fc, W_out, b_out,
           _trace=False):
    from concourse import bass_utils
    c = CFG
    nc = _get_program(c)
    in_maps = prepare_inputs(c, atom_fea, nbr_fea, nbr_fea_idx,
                             crystal_atom_idx, W_emb, b_emb, W_full, b_full,
                             g1, be1, g2, be2, W_fc, b_fc, W_out, b_out)
    res = bass_utils.run_bass_kernel_spmd(
        nc, in_maps, core_ids=list(range(c["NCORES"])), trace=_trace)
    out = np.asarray(res.results[0]["out"], np.float32)
    ret = out[0, :c["NC"]].reshape(c["NC"], 1)
    if _trace:
        return ret, res
    return ret



# revision 12
# speedup vs baseline: 1.6680x; 1.6680x over previous
"""CrystalGraphConvNet forward pass as a distributed Bass/Tile kernel on 8 TRN2
NeuronCores.

Strategy (graph/data parallel, per sharding hint):
  - Atoms sharded contiguously across 8 cores (7500 each, padded to 7552).
  - Per conv layer, each core computes Y_self = x @ Wf[:F], Y_nbr = x @ Wf[F:2F]
    for its atom shard; Y_nbr shards are AllGathered into a replicated f16
    table viewed as PAIR rows (two atoms = 512 B per row, plus a leading zero
    row), so a single int16-indexed dma_gather per tile fetches both parity
    candidates of every edge in one 512 B packet (row index = 1 + j//2).
  - The gather runs un-transposed (contiguous SBUF writes, edge-major):
    ge[p, i, 0:128] / [128:256] hold the even/odd atom of edge i*128+p.  A
    single copy_predicated with a tiny resident per-edge parity mask
    (broadcast along channels) selects the right atom; 12 accumulating
    transpose-matmuls against an f16 identity then fold the selected rows
    into the channel-major PSUM accumulator on the TensorEngine, on top of
    the nbr_fea projection and the one-hot self term.
  - Training-mode batchnorm needs global stats, so gated values are staged to
    DRAM scratch in f16 while per-channel sum/sumsq accumulate (scalar-engine
    copy/square with accum); a tiny AllReduce yields the affine.  The second
    pass is a SINGLE pass per tile pair: sigmoid is computed as
    1/(1+exp(-a)) with a DVE reciprocal, so every activation in the program
    lives in the one exp/ln table set and no ACT table reloads occur.
  - Crystal mean-pooling is one accumulating one-hot matmul per tile into a
    512-crystal window, scattered by int32 indirect DMA into a global crystal
    array and AllReduced; the tiny MLP head runs redundantly on every core.
"""

import math
import os
import numpy as np

import concourse.bass as bass
import concourse.bacc as bacc
import concourse.tile as tile
from concourse import mybir
from contextlib import ExitStack

F16 = mybir.dt.float16
F32 = mybir.dt.float32
I8 = mybir.dt.int8
I16 = mybir.dt.int16
I32 = mybir.dt.int32


def make_cfg(N=60000, M=12, F0=92, FB=41, F=64, H=128, NC=2000, NCONV=3,
             EPS=1e-5, NCORES=8, TA=128):
    c = dict(N=N, M=M, F0=F0, FB=FB, F=F, H=H, NC=NC, NCONV=NCONV, EPS=EPS,
             NCORES=NCORES, TA=TA)
    assert N % NCORES == 0
    c["A_shard"] = N // NCORES
    c["ntile"] = (c["A_shard"] + TA - 1) // TA
    c["A_pad"] = c["ntile"] * TA
    c["E_tile"] = TA * M
    c["E_loc"] = c["ntile"] * c["E_tile"]
    assert (NCORES * c["A_pad"]) % 2 == 0
    c["R2"] = 1 + NCORES * c["A_pad"] // 2      # pair-table rows (zero row at 0)
    assert c["R2"] <= 32768, "pair table must stay int16-addressable"
    c["W16"] = c["E_tile"] // 16
    c["NCB"] = 512 * ((NC + 511) // 512) + 512  # crystal bounce rows
    c["G"] = 2 * F                              # gated channels
    return c


CFG = make_cfg()


# --------------------------------------------------------------------------
# program builder
# --------------------------------------------------------------------------

def build_program(c, debug=False, dbg_dump=False, stop=None):
    # stop: optional (layer, stage) tuple for bisection; stages within a layer:
    # 0=Y, 1=AG, 2=A, 3=AR1, 4=B, 5=AR2, 6=C; (L,0)=pool, (L,1)=head
    nc = bacc.Bacc("TRN2", target_bir_lowering=False, debug=debug,
                   num_devices=c["NCORES"])

    N, M, F0, FB, F, H, NC, L = (c["N"], c["M"], c["F0"], c["FB"], c["F"],
                                 c["H"], c["NC"], c["NCONV"])
    G, TA, NT, AP_, ET, EL = (c["G"], c["TA"], c["ntile"], c["A_pad"],
                              c["E_tile"], c["E_loc"])
    R2, W16, NCB, EPS = c["R2"], c["W16"], c["NCB"], c["EPS"]
    NPAIR = (NT + 1) // 2
    NCHUNK = ET // TA                           # 128-edge chunks per tile

    # ---------------- inputs ----------------
    afT = nc.dram_tensor("afT", [F0, AP_], F16, kind="ExternalInput")
    idxw = nc.dram_tensor("idxw", [128, NT, W16], I16, kind="ExternalInput")
    mparr = nc.dram_tensor("mparr", [128, NT, M], I8, kind="ExternalInput")
    nbrT = nc.dram_tensor("nbrT", [FB, EL], F16, kind="ExternalInput")
    oh_self = nc.dram_tensor("oh_self", [128, ET], F16, kind="ExternalInput")
    pone = nc.dram_tensor("pone", [NT, 128, 512], F16, kind="ExternalInput")
    scidx = nc.dram_tensor("scidx", [128, 4], I32, kind="ExternalInput")
    invcnt = nc.dram_tensor("invcnt", [NCB, 1], F32, kind="ExternalInput")
    w_emb = nc.dram_tensor("w_emb", [F0, F], F16, kind="ExternalInput")
    b_emb = nc.dram_tensor("b_emb", [F, 1], F32, kind="ExternalInput")
    w_self = nc.dram_tensor("w_self", [L, F, G], F16, kind="ExternalInput")
    w_nbr = nc.dram_tensor("w_nbr", [L, F, G], F16, kind="ExternalInput")
    w_b = nc.dram_tensor("w_b", [L, FB, G], F16, kind="ExternalInput")
    g1 = nc.dram_tensor("g1", [L, G, 1], F32, kind="ExternalInput")
    be1 = nc.dram_tensor("be1", [L, G, 1], F32, kind="ExternalInput")
    g2 = nc.dram_tensor("g2", [L, F, 1], F32, kind="ExternalInput")
    be2 = nc.dram_tensor("be2", [L, F, 1], F32, kind="ExternalInput")
    w_fc = nc.dram_tensor("w_fc", [F, H], F16, kind="ExternalInput")
    b_fc = nc.dram_tensor("b_fc", [H, 1], F32, kind="ExternalInput")
    w_out = nc.dram_tensor("w_out", [H, 1], F16, kind="ExternalInput")
    b_out = nc.dram_tensor("b_out", [1, 1], F32, kind="ExternalInput")
    ident = nc.dram_tensor("ident", [128, 128], F32, kind="ExternalInput")
    identh = nc.dram_tensor("identh", [128, 128], F16, kind="ExternalInput")

    out_t = nc.dram_tensor("out", [1, NCB], F32, kind="ExternalOutput")
    if dbg_dump:
        L_, F_, G_, AP2, EL_ = c["NCONV"], c["F"], c["G"], c["A_pad"], c["E_loc"]
        dbgx0 = nc.dram_tensor("dbgx0", [F_, AP2], F32, kind="ExternalOutput")
        dbgx = nc.dram_tensor("dbgx", [L_, F_, AP2], F32, kind="ExternalOutput")
        dbgsum = nc.dram_tensor("dbgsum", [L_, F_, AP2], F32, kind="ExternalOutput")
        dbgst1 = nc.dram_tensor("dbgst1", [L_, G_, 2], F32, kind="ExternalOutput")
        dbggat = nc.dram_tensor("dbggat", [L_, NT, 128, ET], F16,
                                kind="ExternalOutput")
        dbgpool = nc.dram_tensor("dbgpool", [NCB, F_], F32, kind="ExternalOutput")

    # ---------------- internal DRAM ----------------
    yb = nc.dram_tensor("yb", [AP_, G], F16)                        # AG input bounce
    tbl = nc.dram_tensor("tbl", [R2, 2 * G], F16, addr_space="Shared")
    scr = nc.dram_tensor("scr", [NT, 128, ET], F16)
    ar1_in = nc.dram_tensor("ar1_in", [G, 2], F32)
    ar1_out = nc.dram_tensor("ar1_out", [G, 2], F32, addr_space="Shared")
    ar2_in = nc.dram_tensor("ar2_in", [F, 2], F32)
    ar2_out = nc.dram_tensor("ar2_out", [F, 2], F32, addr_space="Shared")
    pool_in = nc.dram_tensor("pool_in", [NCB, F], F32)
    pool_out = nc.dram_tensor("pool_out", [NCB, F], F32, addr_space="Shared")

    rg = [list(range(c["NCORES"]))]
    AF = mybir.ActivationFunctionType
    OP = mybir.AluOpType
    if stop is None:
        stop = (L, 9)

    def live(key):
        return key <= stop

    with tile.TileContext(nc) as tc, ExitStack() as top:
        # persistent SBUF state
        x_cm = nc.alloc_sbuf_tensor("x_cm", [F, AP_], F32)
        summed = nc.alloc_sbuf_tensor("summed", [F, AP_], F16)
        ysr = nc.alloc_sbuf_tensor("ysr", [128, NT, G], F16)      # Y_self row-major
        idx_all = nc.alloc_sbuf_tensor("idx_all", [128, NT, W16], I16)
        mpar = nc.alloc_sbuf_tensor("mpar", [128, NT, M, 1], I8)

        const = top.enter_context(tc.tile_pool(name="const", bufs=1))
        stats = top.enter_context(tc.tile_pool(name="stats", bufs=1))

        # constants resident all kernel
        ohs_t = const.tile([128, ET], F16)
        nc.sync.dma_start(out=ohs_t[:], in_=oh_self[:, :])
        wemb_t = const.tile([F0, F], F16)
        nc.sync.dma_start(out=wemb_t[:], in_=w_emb[:, :])
        bemb_t = const.tile([F, 1], F32)
        nc.sync.dma_start(out=bemb_t[:], in_=b_emb[:, :])
        id_t = const.tile([128, 128], F32)
        nc.sync.dma_start(out=id_t[:], in_=ident[:, :])
        idh_t = const.tile([128, 128], F16)
        nc.sync.dma_start(out=idh_t[:], in_=identh[:, :])
        eps_t = const.tile([128, 1], F32)
        nc.vector.memset(eps_t[:], EPS)
        zrow = const.tile([1, 2 * G], F16)
        nc.vector.memset(zrow[:], 0.0)
        zt128 = const.tile([128, F], F32)
        nc.vector.memset(zt128[:], 0.0)

        # layer-invariant gather indices and parity masks
        nc.sync.dma_start(out=idx_all[:, :, :], in_=idxw[:, :, :])
        nc.sync.dma_start(out=mpar[:, :, :, 0], in_=mparr[:, :, :])

        wS = []
        wN = []
        wB = []
        for l in range(L):
            t1 = const.tile([F, G], F16, tag=f"wS{l}")
            nc.sync.dma_start(out=t1[:], in_=w_self[l, :, :])
            t2 = const.tile([F, G], F16, tag=f"wN{l}")
            nc.sync.dma_start(out=t2[:], in_=w_nbr[l, :, :])
            t3 = const.tile([FB, G], F16, tag=f"wB{l}")
            nc.sync.dma_start(out=t3[:], in_=w_b[l, :, :])
            wS.append(t1)
            wN.append(t2)
            wB.append(t3)

        # stats buffers
        st1_s = stats.tile([G, NT], F32, tag="st1s")
        st1_q = stats.tile([G, NT], F32, tag="st1q")
        st2_s = stats.tile([F, 2 * NPAIR], F32, tag="st2s")
        st2_q = stats.tile([F, 2 * NPAIR], F32, tag="st2q")

        # zero table guard row + summed pads
        nc.sync.dma_start(out=tbl[0:1, :], in_=zrow[:])
        nc.vector.memset(summed[:, :], 0.0)

        # ---------------- embedding: x = atom_fea @ W_emb + b_emb ----------
        with tc.tile_pool(name="emb", bufs=3) as embp, \
             tc.tile_pool(name="embps", bufs=2, space="PSUM") as embps:
            CH = 512
            for j in range(0, AP_, CH):
                w = min(CH, AP_ - j)
                rhs = embp.tile([F0, CH], F16, tag="embr")
                nc.sync.dma_start(out=rhs[:, :w], in_=afT[:, j:j + w])
                ps = embps.tile([F, CH], F32, tag="embp")
                nc.tensor.matmul(ps[:, :w], lhsT=wemb_t[:], rhs=rhs[:, :w],
                                 start=True, stop=True)
                nc.scalar.activation(out=x_cm[:, j:j + w], in_=ps[:, :w],
                                     func=AF.Identity, bias=bemb_t[:], scale=1.0)
        if dbg_dump:
            nc.sync.dma_start(out=dbgx0[:, :], in_=x_cm[:, :])

        # ---------------- conv layers ----------------
        for l in range(L):
            if not live((l, 0)):
                break
            # ---- phase Y: Y_self (SBUF) / Y_nbr (-> bounce -> AllGather) ----
            with tc.tile_pool(name="yph", bufs=3) as yp, \
                 tc.tile_pool(name="yps", bufs=2, space="PSUM") as yps:
                lastreal = c["A_shard"] - (NT - 1) * TA
                for t in range(NT):
                    xa = yp.tile([F, TA], F16, tag="xa")
                    nc.scalar.activation(out=xa[:], in_=x_cm[:, t * TA:(t + 1) * TA],
                                         func=AF.Copy)
                    psS = yps.tile([TA, G], F32, tag="psS")
                    nc.tensor.matmul(psS[:], lhsT=xa[:], rhs=wS[l][:],
                                     start=True, stop=True)
                    # pad atoms of the last tile must contribute exactly zero
                    # through the self one-hot matmul
                    nreal = TA if t < NT - 1 else lastreal
                    if nreal < TA:
                        nc.vector.memset(ysr[:, t, :], 0.0)
                    nc.scalar.activation(out=ysr[0:nreal, t, :],
                                         in_=psS[0:nreal, :], func=AF.Copy)
                    psN = yps.tile([TA, G], F32, tag="psN")
                    nc.tensor.matmul(psN[:], lhsT=xa[:], rhs=wN[l][:],
                                     start=True, stop=True)
                    yn = yp.tile([TA, G], F16, tag="yn")
                    nc.scalar.activation(out=yn[:], in_=psN[:], func=AF.Copy)
                    nc.sync.dma_start(out=yb[t * TA:(t + 1) * TA, :], in_=yn[:])

            if not live((l, 1)):
                break
            tc.strict_bb_all_engine_barrier()
            nc.gpsimd.collective_compute(
                "AllGather", OP.bypass, replica_groups=rg,
                ins=[yb[:, :]], outs=[tbl[1:R2, :]])
            tc.strict_bb_all_engine_barrier()

            if not live((l, 2)):
                break
            # ---- pass A: edges -> gated scratch + stats1 ----
            with tc.tile_pool(name="pa", bufs=4) as pa, \
                 tc.tile_pool(name="paps", bufs=2, space="PSUM") as paps:
                for t in range(NT):
                    nbt = pa.tile([FB, ET], F16, tag="nbt")
                    nc.sync.dma_start(out=nbt[:], in_=nbrT[:, t * ET:(t + 1) * ET])

                    ge = pa.tile([128, NCHUNK, 2 * G], F16, tag="ge")
                    nc.gpsimd.dma_gather(ge[:], tbl[:, :], idx_all[:, t, :],
                                         ET, ET, 2 * G, single_packet=False)
                    # parity select: overwrite even-atom slab with odd-atom
                    # slab wherever the edge's target index is odd
                    nc.vector.copy_predicated(
                        out=ge[:, :, 0:G],
                        mask=mpar[:, t, :, :].broadcast_to([128, NCHUNK, G]),
                        data=ge[:, :, G:2 * G])

                    # every 128-col region: chunk transpose (start) -> wB ->
                    # one-hot self term (stop)
                    ps = paps.tile([G, ET], F32, tag="aps")
                    for i in range(NCHUNK):
                        cs = slice(i * 128, (i + 1) * 128)
                        nc.tensor.matmul(ps[:, cs], lhsT=ge[:, i, 0:G],
                                         rhs=idh_t[:], start=(i % 4 == 0),
                                         stop=False)
                    for s in range(ET // 512):
                        sl = slice(s * 512, (s + 1) * 512)
                        nc.tensor.matmul(ps[:, sl], lhsT=wB[l][:], rhs=nbt[:, sl],
                                         start=False, stop=False)
                        nc.tensor.matmul(ps[:, sl], lhsT=ysr[:, t, :],
                                         rhs=ohs_t[:, sl], start=False, stop=True)

                    gat = pa.tile([128, ET], F16, tag="gat")
                    nc.scalar.activation(out=gat[:], in_=ps[:], func=AF.Copy,
                                         accum_out=st1_s[:, t:t + 1])
                    sqd = pa.tile([128, ET], F16, tag="sqd")
                    nc.scalar.activation(out=sqd[:], in_=ps[:], func=AF.Square,
                                         accum_out=st1_q[:, t:t + 1])
                    nc.sync.dma_start(out=scr[t, :, :], in_=gat[:])

            if not live((l, 3)):
                break
            # ---- stats1 reduce + AllReduce + affine ----
            with tc.tile_pool(name="s1", bufs=1) as s1p:
                pack1 = s1p.tile([G, 2], F32, tag="pack1")
                nc.vector.tensor_reduce(out=pack1[:, 0:1], in_=st1_s[:],
                                        axis=mybir.AxisListType.X, op=OP.add)
                nc.vector.tensor_reduce(out=pack1[:, 1:2], in_=st1_q[:],
                                        axis=mybir.AxisListType.X, op=OP.add)
                nc.sync.dma_start(out=ar1_in[:, :], in_=pack1[:])
                tc.strict_bb_all_engine_barrier()
                nc.gpsimd.collective_compute(
                    "AllReduce", OP.add, replica_groups=rg,
                    ins=[ar1_in[:, :]], outs=[ar1_out[:, :]])
                tc.strict_bb_all_engine_barrier()

                red1 = s1p.tile([G, 2], F32, tag="red1")
                nc.sync.dma_start(out=red1[:], in_=ar1_out[:, :])
                if dbg_dump:
                    nc.sync.dma_start(out=dbgst1[l, :, :], in_=red1[:])
                    for t in range(NT):
                        nc.gpsimd.dma_start(out=dbggat[l, t, :, :],
                                            in_=scr[t, :, :])
                g1_t = s1p.tile([G, 1], F32, tag="g1t")
                nc.sync.dma_start(out=g1_t[:], in_=g1[l, :, :])
                be1_t = s1p.tile([G, 1], F32, tag="be1t")
                nc.sync.dma_start(out=be1_t[:], in_=be1[l, :, :])

                # rsqrt(var+eps) = exp(-0.5*ln(var+eps)) + one Newton step
                # (no sqrt table needed; Ln/Exp share one ACT table set)
                invE = 1.0 / float(N * M)
                mmt = s1p.tile([G, 8], F32, tag="mmt")
                mcol = mmt[:, 0:1]
                nc.vector.tensor_scalar(out=mcol, in0=red1[:, 0:1], scalar1=invE,
                                        scalar2=None, op0=OP.mult)
                ex2 = mmt[:, 1:2]
                nc.vector.tensor_scalar(out=ex2, in0=red1[:, 1:2], scalar1=invE,
                                        scalar2=None, op0=OP.mult)
                msq = mmt[:, 2:3]
                nc.vector.tensor_tensor(out=msq, in0=mcol, in1=mcol, op=OP.mult)
                var = mmt[:, 3:4]
                nc.vector.tensor_tensor(out=var, in0=ex2, in1=msq, op=OP.subtract)
                lv = mmt[:, 4:5]
                nc.scalar.activation(out=lv, in_=var, func=AF.Ln,
                                     bias=eps_t[0:G, :], scale=1.0)
                r0 = mmt[:, 5:6]
                nc.scalar.activation(out=r0, in_=lv, func=AF.Exp, scale=-0.5)
                # one Newton step: r1 = r0*(1.5 - 0.5*(var+eps)*r0^2)
                vpe = mmt[:, 6:7]
                nc.vector.tensor_scalar(out=vpe, in0=var, scalar1=eps_t[0:G, :],
                                        scalar2=0.5, op0=OP.add, op1=OP.mult)
                r0q = mmt[:, 7:8]
                nc.vector.tensor_tensor(out=r0q, in0=r0, in1=r0, op=OP.mult)
                nc.vector.tensor_tensor(out=r0q, in0=r0q, in1=vpe, op=OP.mult)
                nc.vector.tensor_scalar(out=r0q, in0=r0q, scalar1=-1.0,
                                        scalar2=1.5, op0=OP.mult, op1=OP.add)
                r1 = mmt[:, 6:7]
                nc.vector.tensor_tensor(out=r1, in0=r0, in1=r0q, op=OP.mult)

                s1c_ = s1p.tile([G, 1], F32, tag="s1c")
                nc.vector.tensor_tensor(out=s1c_[:], in0=g1_t[:], in1=r1,
                                        op=OP.mult)
                t1c_ = s1p.tile([G, 1], F32, tag="t1c")
                nc.vector.tensor_tensor(out=t1c_[:], in0=mcol, in1=s1c_[:],
                                        op=OP.mult)
                nc.vector.scalar_tensor_tensor(out=t1c_[:], in0=t1c_[:],
                                               scalar=-1.0, in1=be1_t[:],
                                               op0=OP.mult, op1=OP.add)
                # negated F-half affine (sigmoid via 1/(1+exp(-a)))
                s1n = s1p.tile([G, 1], F32, tag="s1n")
                nc.vector.tensor_scalar(out=s1n[:], in0=s1c_[:], scalar1=-1.0,
                                        scalar2=None, op0=OP.mult)
                t1n = s1p.tile([G, 1], F32, tag="t1n")
                nc.vector.tensor_scalar(out=t1n[:], in0=t1c_[:], scalar1=-1.0,
                                        scalar2=None, op0=OP.mult)
                # replicated (packed-pair) scale/bias
                sF = s1p.tile([128, 1], F32, tag="sF")
                tF = s1p.tile([128, 1], F32, tag="tF")
                sC = s1p.tile([128, 1], F32, tag="sC")
                tC = s1p.tile([128, 1], F32, tag="tC")
                for half in range(2):
                    hp = slice(half * F, half * F + F)
                    nc.sync.dma_start(out=sF[hp, :], in_=s1n[0:F, :])
                    nc.sync.dma_start(out=tF[hp, :], in_=t1n[0:F, :])
                    nc.sync.dma_start(out=sC[hp, :], in_=s1c_[F:G, :])
                    nc.sync.dma_start(out=tC[hp, :], in_=t1c_[F:G, :])

                if not live((l, 4)):
                    break
                # ---- pass B: sigmoid*softplus, neighbor-sum, stats2 ----
                tc.strict_bb_all_engine_barrier()
                with tc.tile_pool(name="pb", bufs=3) as bp:
                    for p in range(NPAIR):
                        t0, t1_ = 2 * p, min(2 * p + 1, NT - 1)
                        single = (2 * p + 1 > NT - 1)
                        pp = 128 if not single else F
                        zf = bp.tile([128, ET], F16, tag="zf")
                        zc = bp.tile([128, ET], F16, tag="zc")
                        nc.sync.dma_start(out=zf[0:F, :], in_=scr[t0, 0:F, :])
                        nc.sync.dma_start(out=zc[0:F, :], in_=scr[t0, F:G, :])
                        if not single:
                            nc.sync.dma_start(out=zf[F:G, :], in_=scr[t1_, 0:F, :])
                            nc.sync.dma_start(out=zc[F:G, :], in_=scr[t1_, F:G, :])
                        # ef = exp(-(sF*zf+tF)); inf-safe: sigma = 1/(1+ef)
                        ef = bp.tile([128, ET], F16, tag="ef")
                        nc.scalar.activation(out=ef[0:pp, :], in_=zf[0:pp, :],
                                             func=AF.Exp,
                                             bias=tF[0:pp, :], scale=sF[0:pp, :])
                        ec = bp.tile([128, ET], F32, tag="ec")
                        nc.scalar.activation(out=ec[0:pp, :], in_=zc[0:pp, :],
                                             func=AF.Exp,
                                             bias=tC[0:pp, :], scale=sC[0:pp, :])
                        sp = bp.tile([128, ET], F16, tag="sp")
                        nc.scalar.activation(out=sp[0:pp, :], in_=ec[0:pp, :],
                                             func=AF.Ln, bias=1.0, scale=1.0)
                        nc.vector.tensor_scalar(out=ef[0:pp, :], in0=ef[0:pp, :],
                                                scalar1=1.0, scalar2=None,
                                                op0=OP.add)
                        with nc.allow_low_precision("sigmoid recip in f16"):
                            nc.vector.reciprocal(out=ef[0:pp, :], in_=ef[0:pp, :])
                        nc.vector.tensor_tensor(out=sp[0:pp, :], in0=sp[0:pp, :],
                                                in1=ef[0:pp, :], op=OP.mult)
                        halves = 1 if single else 2
                        for hh in range(halves):
                            tt = t0 if hh == 0 else t1_
                            hs = slice(hh * F, hh * F + F)
                            red = bp.tile([F, TA], F32, tag="red")
                            nc.vector.tensor_reduce(
                                out=red[:],
                                in_=sp[hs, :].rearrange("p (a m) -> p a m", m=M),
                                axis=mybir.AxisListType.X, op=OP.add)
                            nc.scalar.activation(
                                out=summed[:, tt * TA:(tt + 1) * TA],
                                in_=red[:], func=AF.Copy)
                            # stats over real atoms only
                            nreal = min(c["A_shard"] - tt * TA, TA)
                            col = 2 * p + hh
                            nc.vector.tensor_reduce(
                                out=st2_s[:, col:col + 1], in_=red[:, 0:nreal],
                                axis=mybir.AxisListType.X, op=OP.add)
                            sqt = bp.tile([F, TA], F32, tag="sqt")
                            nc.scalar.activation(
                                out=sqt[:, 0:nreal], in_=red[:, 0:nreal],
                                func=AF.Square,
                                accum_out=st2_q[:, col:col + 1])

                if not live((l, 5)):
                    break
                # ---- stats2 AllReduce + affine2 + pass C ----
                ncols = NT
                pack2 = s1p.tile([F, 2], F32, tag="pack2")
                nc.vector.tensor_reduce(out=pack2[:, 0:1],
                                        in_=st2_s[:, 0:ncols],
                                        axis=mybir.AxisListType.X, op=OP.add)
                nc.vector.tensor_reduce(out=pack2[:, 1:2],
                                        in_=st2_q[:, 0:ncols],
                                        axis=mybir.AxisListType.X, op=OP.add)
                nc.sync.dma_start(out=ar2_in[:, :], in_=pack2[:])
                tc.strict_bb_all_engine_barrier()
                nc.gpsimd.collective_compute(
                    "AllReduce", OP.add, replica_groups=rg,
                    ins=[ar2_in[:, :]], outs=[ar2_out[:, :]])
                tc.strict_bb_all_engine_barrier()

                red2 = s1p.tile([F, 2], F32, tag="red2")
                nc.sync.dma_start(out=red2[:], in_=ar2_out[:, :])
                g2_t = s1p.tile([F, 1], F32, tag="g2t")
                nc.sync.dma_start(out=g2_t[:], in_=g2[l, :, :])
                be2_t = s1p.tile([F, 1], F32, tag="be2t")
                nc.sync.dma_start(out=be2_t[:], in_=be2[l, :, :])

                invN = 1.0 / float(N)
                mt2 = s1p.tile([F, 8], F32, tag="mt2")
                m2c = mt2[:, 0:1]
                nc.vector.tensor_scalar(out=m2c, in0=red2[:, 0:1], scalar1=invN,
                                        scalar2=None, op0=OP.mult)
                e2c = mt2[:, 1:2]
                nc.vector.tensor_scalar(out=e2c, in0=red2[:, 1:2], scalar1=invN,
                                        scalar2=None, op0=OP.mult)
                ms2 = mt2[:, 2:3]
                nc.vector.tensor_tensor(out=ms2, in0=m2c, in1=m2c, op=OP.mult)
                v2 = mt2[:, 3:4]
                nc.vector.tensor_tensor(out=v2, in0=e2c, in1=ms2, op=OP.subtract)
                lv2 = mt2[:, 4:5]
                nc.scalar.activation(out=lv2, in_=v2, func=AF.Ln,
                                     bias=eps_t[0:F, :], scale=1.0)
                r02 = mt2[:, 5:6]
                nc.scalar.activation(out=r02, in_=lv2, func=AF.Exp, scale=-0.5)
                vpe2 = mt2[:, 6:7]
                nc.vector.tensor_scalar(out=vpe2, in0=v2, scalar1=eps_t[0:F, :],
                                        scalar2=0.5, op0=OP.add, op1=OP.mult)
                r0q2 = mt2[:, 7:8]
                nc.vector.tensor_tensor(out=r0q2, in0=r02, in1=r02, op=OP.mult)
                nc.vector.tensor_tensor(out=r0q2, in0=r0q2, in1=vpe2, op=OP.mult)
                nc.vector.tensor_scalar(out=r0q2, in0=r0q2, scalar1=-1.0,
                                        scalar2=1.5, op0=OP.mult, op1=OP.add)
                r12 = mt2[:, 6:7]
                nc.vector.tensor_tensor(out=r12, in0=r02, in1=r0q2, op=OP.mult)
                s2c = s1p.tile([F, 1], F32, tag="s2c")
                nc.vector.tensor_tensor(out=s2c[:], in0=g2_t[:], in1=r12,
                                        op=OP.mult)
                t2c = s1p.tile([F, 1], F32, tag="t2c")
                nc.vector.tensor_tensor(out=t2c[:], in0=m2c, in1=s2c[:],
                                        op=OP.mult)
                nc.vector.scalar_tensor_tensor(out=t2c[:], in0=t2c[:],
                                               scalar=-1.0, in1=be2_t[:],
                                               op0=OP.mult, op1=OP.add)

                if not live((l, 6)):
                    break
                # pass C: x = softplus(x + s2*summed + t2) via ln(1+exp)
                with tc.tile_pool(name="pc", bufs=2) as pcp:
                    CW = 2048
                    for j in range(0, AP_, CW):
                        w = min(CW, AP_ - j)
                        pre = pcp.tile([F, CW], F32, tag="pre")
                        nc.vector.scalar_tensor_tensor(
                            out=pre[:, :w], in0=summed[:, j:j + w],
                            scalar=s2c[:, 0:1], in1=x_cm[:, j:j + w],
                            op0=OP.mult, op1=OP.add)
                        pex = pcp.tile([F, CW], F32, tag="pex")
                        nc.scalar.activation(out=pex[:, :w], in_=pre[:, :w],
                                             func=AF.Exp, bias=t2c[:], scale=1.0)
                        nc.scalar.activation(out=x_cm[:, j:j + w], in_=pex[:, :w],
                                             func=AF.Ln, bias=1.0, scale=1.0)
                if dbg_dump:
                    nc.gpsimd.dma_start(out=dbgsum[l, :, :], in_=summed[:, :])
                    nc.sync.dma_start(out=dbgx[l, :, :], in_=x_cm[:, :])

        for _ in range(1 if live((L, 0)) else 0):
            # ---------------- crystal pooling ----------------
            with tc.tile_pool(name="pool", bufs=4) as pp_, \
                 tc.tile_pool(name="poolacc", bufs=1, space="PSUM") as pacc, \
                 tc.tile_pool(name="poolps", bufs=2, space="PSUM") as pps:
                acc = pacc.tile([F, 512], F32, tag="pacc")
                for t in range(NT):
                    xps = pps.tile([TA, F], F32, tag="xps")
                    nc.tensor.transpose(out=xps[:], in_=x_cm[:, t * TA:(t + 1) * TA],
                                        identity=id_t[0:F, 0:F])
                    xrm = pp_.tile([TA, F], F16, tag="xrm")
                    nc.scalar.activation(out=xrm[:], in_=xps[:], func=AF.Copy)
                    oht = pp_.tile([128, 512], F16, tag="oht")
                    nc.sync.dma_start(out=oht[:], in_=pone[t, :, :])
                    nc.tensor.matmul(acc[:], lhsT=xrm[:], rhs=oht[:],
                                     start=(t == 0), stop=(t == NT - 1))
                # zero pool_in, then scatter our window rows
                for r_ in range(NCB // 128):
                    nc.sync.dma_start(out=pool_in[128 * r_:128 * (r_ + 1), :],
                                      in_=zt128[:, 0:F])
                sci = pp_.tile([128, 4], I32, tag="sci")
                nc.sync.dma_start(out=sci[:], in_=scidx[:, :])
                tc.strict_bb_all_engine_barrier()
                accS = pp_.tile([F, 512], F32, tag="accS")
                nc.vector.tensor_copy(out=accS[:], in_=acc[:])
                accs = []
                for ci in range(4):
                    # [F, 128] crystal block -> row-major [128, F]
                    tps = pps.tile([128, F], F32, tag="tps")
                    nc.tensor.transpose(out=tps[:],
                                        in_=accS[:, ci * 128:(ci + 1) * 128],
                                        identity=id_t[0:F, 0:F])
                    asb = pp_.tile([128, F], F32, tag=f"asb{ci}")
                    nc.vector.tensor_copy(out=asb[:], in_=tps[:])
                    accs.append(asb)
                for ci in range(4):
                    nc.gpsimd.indirect_dma_start(
                        out=pool_in[:, :],
                        out_offset=bass.IndirectOffsetOnAxis(ap=sci[:, ci:ci + 1], axis=0),
                        in_=accs[ci][:], in_offset=None)
                tc.strict_bb_all_engine_barrier()
                nc.gpsimd.collective_compute(
                    "AllReduce", OP.add, replica_groups=rg,
                    ins=[pool_in[:, :]], outs=[pool_out[:, :]])
                tc.strict_bb_all_engine_barrier()

            if dbg_dump:
                nc.sync.dma_start(out=dbgpool[:, :], in_=pool_out[:, :])


        for _ in range(1 if live((L, 1)) else 0):
            # ---------------- head ----------------
            with tc.tile_pool(name="head", bufs=3) as hp, \
                 tc.tile_pool(name="headps", bufs=2, space="PSUM") as hps:
                crys = nc.alloc_sbuf_tensor("crys", [F, NCB], F16)
                wfc_t = hp.tile([F, H], F16, tag="wfc")
                nc.sync.dma_start(out=wfc_t[:], in_=w_fc[:, :])
                bfc_t = hp.tile([H, 1], F32, tag="bfc")
                nc.sync.dma_start(out=bfc_t[:], in_=b_fc[:, :])
                wout_t = hp.tile([H, 1], F16, tag="wout")
                nc.sync.dma_start(out=wout_t[:], in_=w_out[:, :])
                bout_t = hp.tile([1, 1], F32, tag="bout")
                nc.sync.dma_start(out=bout_t[:], in_=b_out[:, :])

                for r_ in range(NCB // 128):
                    pt = hp.tile([128, F], F32, tag="pt")
                    nc.sync.dma_start(out=pt[:], in_=pool_out[128 * r_:128 * (r_ + 1), :])
                    ic = hp.tile([128, 1], F32, tag="ic")
                    nc.sync.dma_start(out=ic[:], in_=invcnt[128 * r_:128 * (r_ + 1), :])
                    nc.vector.tensor_scalar(out=pt[:], in0=pt[:], scalar1=ic[:, 0:1],
                                            scalar2=None, op0=OP.mult)
                    pex2 = hp.tile([128, F], F32, tag="pex2")
                    nc.scalar.activation(out=pex2[:], in_=pt[:], func=AF.Exp)
                    spt = hp.tile([128, F], F32, tag="spt")
                    nc.scalar.activation(out=spt[:], in_=pex2[:], func=AF.Ln,
                                         bias=1.0, scale=1.0)
                    tps = hps.tile([F, 128], F32, tag="tps")
                    nc.tensor.transpose(out=tps[:], in_=spt[:], identity=id_t[:, :])
                    nc.scalar.activation(out=crys[:, 128 * r_:128 * (r_ + 1)],
                                         in_=tps[:], func=AF.Copy)

                hc = nc.alloc_sbuf_tensor("hc", [H, NCB], F16)
                for j in range(0, NCB, 512):
                    psh = hps.tile([H, 512], F32, tag="psh")
                    nc.tensor.matmul(psh[:], lhsT=wfc_t[:], rhs=crys[:, j:j + 512],
                                     start=True, stop=True)
                    hex_ = hp.tile([H, 512], F32, tag="hex")
                    nc.scalar.activation(out=hex_[:], in_=psh[:],
                                         func=AF.Exp, bias=bfc_t[:], scale=1.0)
                    nc.scalar.activation(out=hc[:, j:j + 512], in_=hex_[:],
                                         func=AF.Ln, bias=1.0, scale=1.0)
                ofin = hp.tile([1, NCB], F32, tag="ofin")
                for j in range(0, NCB, 512):
                    pso = hps.tile([1, 512], F32, tag="pso")
                    nc.tensor.matmul(pso[:], lhsT=wout_t[:], rhs=hc[:, j:j + 512],
                                     start=True, stop=True)
                    nc.scalar.activation(out=ofin[:, j:j + 512], in_=pso[:],
                                         func=AF.Identity, bias=bout_t[:], scale=1.0)
                nc.sync.dma_start(out=out_t[:, :], in_=ofin[:])

    nc.compile()
    return nc


# --------------------------------------------------------------------------
# host-side input preparation
# --------------------------------------------------------------------------

def prepare_inputs(c, atom_fea, nbr_fea, nbr_fea_idx, crystal_atom_idx,
                   W_emb, b_emb, W_full, b_full, g1, be1, g2, be2,
                   W_fc, b_fc, W_out, b_out):
    N, M, F0, FB, F, H, NC, L = (c["N"], c["M"], c["F0"], c["FB"], c["F"],
                                 c["H"], c["NC"], c["NCONV"])
    G, TA, NT, AP_, ET, EL = (c["G"], c["TA"], c["ntile"], c["A_pad"],
                              c["E_tile"], c["E_loc"])
    W16, NCB, K = c["W16"], c["NCB"], c["NCORES"]
    AS = c["A_shard"]

    atom_fea = np.asarray(atom_fea, np.float32)
    nbr_fea = np.asarray(nbr_fea, np.float32)
    nbr_fea_idx = np.asarray(nbr_fea_idx, np.int64)
    crystal_atom_idx = np.asarray(crystal_atom_idx, np.int64)

    # shared (replicated) tensors
    oh = np.zeros((128, ET), np.float16)
    for j in range(ET):
        oh[j // M, j] = 1.0
    shared = {
        "oh_self": oh,
        "w_emb": np.asarray(W_emb, np.float16),
        "b_emb": np.asarray(b_emb, np.float32).reshape(F, 1),
        "w_self": np.asarray(W_full[:, :F, :], np.float16),
        "w_nbr": np.asarray(W_full[:, F:2 * F, :], np.float16),
        "w_b": np.asarray(W_full[:, 2 * F:, :], np.float16),
        "g1": np.asarray(g1, np.float32).reshape(L, G, 1),
        "be1": np.asarray(be1, np.float32).reshape(L, G, 1),
        "g2": np.asarray(g2, np.float32).reshape(L, F, 1),
        "be2": np.asarray(be2, np.float32).reshape(L, F, 1),
        "w_fc": np.asarray(W_fc, np.float16),
        "b_fc": np.asarray(b_fc, np.float32).reshape(H, 1),
        "w_out": np.asarray(W_out, np.float16),
        "b_out": np.asarray(b_out, np.float32).reshape(1, 1),
        "ident": np.eye(128, dtype=np.float32),
        "identh": np.eye(128, dtype=np.float16),
    }
    # crystal counts (global, from index data only)
    cnt = np.bincount(crystal_atom_idx, minlength=NC).astype(np.float32)
    icnt = np.zeros((NCB, 1), np.float32)
    icnt[:NC, 0] = 1.0 / np.maximum(cnt, 1.0)
    shared["invcnt"] = icnt

    # b_full is mathematically irrelevant (cancelled by training-mode BN)

    in_maps = []
    for k in range(K):
        a0 = k * AS
        af = np.zeros((F0, AP_), np.float16)
        af[:, :AS] = atom_fea[a0:a0 + AS].T
        # edge ordering: e = a*M + m within each tile of TA atoms
        gi = np.zeros((NT * TA, M), np.int64)
        gi_raw = nbr_fea_idx[a0:a0 + AS]
        gi[:AS] = gi_raw
        valid = np.zeros((NT * TA, M), bool)
        valid[:AS] = True
        jj = (gi // AS) * AP_ + (gi % AS)          # padded-global atom index
        rows = np.where(valid, 1 + jj // 2, 0).astype(np.int64)
        par = np.where(valid, jj & 1, 0).astype(np.int64)
        idxw = np.zeros((128, NT, W16), np.int16)
        mparr = np.zeros((128, NT, M), np.int8)
        j = np.arange(ET)
        for t in range(NT):
            fl = rows[t * TA:(t + 1) * TA].reshape(ET)
            wrap = np.zeros((16, W16), np.int16)
            wrap[j % 16, j // 16] = fl
            idxw[:, t, :] = np.tile(wrap, (8, 1))
            # ge[p, i, :] holds edge i*128+p -> mask[p, i]
            mparr[:, t, :] = par[t * TA:(t + 1) * TA].reshape(ET)[
                (np.arange(M)[None, :] * 128 + np.arange(128)[:, None])]
        nb = np.zeros((FB, EL), np.float16)
        nb_l = nbr_fea[a0:a0 + AS].reshape(AS * M, FB)
        src = np.zeros((NT * TA * M, FB), np.float32)
        src[:AS * M] = nb_l
        # src is already in (a, m) order; tiles are contiguous runs of ET
        nb[:, :] = src.T.astype(np.float16)

        cry = np.zeros(NT * TA, np.int64)
        cry[:AS] = crystal_atom_idx[a0:a0 + AS]
        cb = int(crystal_atom_idx[a0:a0 + AS].min())
        cmax = int(crystal_atom_idx[a0:a0 + AS].max())
        assert cmax - cb < 512, f"crystal window too wide: {cmax - cb}"
        # pool one-hot: [atom (partition), crystal-window col]
        pone = np.zeros((NT, TA, 512), np.float16)
        for t in range(NT):
            for a in range(TA):
                ga = t * TA + a
                if ga >= AS:
                    continue
                pone[t, a, int(cry[ga]) - cb] = 1.0
        scidx = (cb + np.arange(512, dtype=np.int32)).reshape(4, 128).T.copy()
        assert cb + 512 <= NCB

        in_maps.append(dict(shared,
                            afT=af, idxw=idxw, mparr=mparr, nbrT=nb, pone=pone,
                            scidx=scidx))
    return in_maps


# --------------------------------------------------------------------------
# public entry point
# --------------------------------------------------------------------------

_PROG_CACHE = {}


def _get_program(c):
    key = tuple(sorted((k, v) for k, v in c.items()))
    if key not in _PROG_CACHE:
        _PROG_CACHE[key] = build_program(c)
    return _PROG_CACHE[key]


def kernel(atom_fea, nbr_fea, nbr_fea_idx, crystal_atom_idx, W_emb, b_emb,
           W_full, b_full, g1, be1, g2, be2, W_fc, b_fc, W_out, b_out,
           _trace=False):
    from concourse import bass_utils
    c = CFG
    nc = _get_program(c)
    in_maps = prepare_inputs(c, atom_fea, nbr_fea, nbr_fea_idx,
                             crystal_atom_idx, W_emb, b_emb, W_full, b_full,
                             g1, be1, g2, be2, W_fc, b_fc, W_out, b_out)
    res = bass_utils.run_bass_kernel_spmd(
        nc, in_maps, core_ids=list(range(c["NCORES"])), trace=_trace)
    out = np.asarray(res.results[0]["out"], np.float32)
    ret = out[0, :c["NC"]].reshape(c["NC"], 1)
    if _trace:
        return ret, res
    return ret


# revision 14
# speedup vs baseline: 1.9692x; 1.1806x over previous
"""CrystalGraphConvNet forward pass as a distributed Bass/Tile kernel on 8 TRN2
NeuronCores.

Strategy (graph/data parallel, per sharding hint):
  - Atoms sharded contiguously across 8 cores (7500 each, padded to 7552).
  - Per conv layer, each core computes Y_self = x @ Wf[:F], Y_nbr = x @ Wf[F:2F]
    for its atom shard; Y_nbr shards are AllGathered into a replicated f16
    table viewed as PAIR rows (two atoms = 512 B per row, plus a leading zero
    row), so a single int16-indexed dma_gather per tile fetches both parity
    candidates of every edge in one 512 B packet (row index = 1 + j//2).
  - The gather runs un-transposed (contiguous SBUF writes, edge-major):
    ge[p, i, 0:128] / [128:256] hold the even/odd atom of edge i*128+p.  A
    single copy_predicated with a tiny resident per-edge parity mask
    (broadcast along channels) selects the right atom; 12 accumulating
    transpose-matmuls against an f16 identity then fold the selected rows
    into the channel-major PSUM accumulator on the TensorEngine, on top of
    the nbr_fea projection and the one-hot self term.
  - Training-mode batchnorm needs global stats, so gated values are staged to
    DRAM scratch in f16 while per-channel sum/sumsq accumulate (scalar-engine
    copy/square with accum); a tiny AllReduce yields the affine.  The second
    pass is a SINGLE pass per tile pair: sigmoid is computed as
    1/(1+exp(-a)) with a DVE reciprocal, so every activation in the program
    lives in the one exp/ln table set and no ACT table reloads occur.
  - Crystal mean-pooling is one accumulating one-hot matmul per tile into a
    512-crystal window, scattered by int32 indirect DMA into a global crystal
    array and AllReduced; the tiny MLP head runs redundantly on every core.
"""

import math
import os
import numpy as np

import concourse.bass as bass
import concourse.bacc as bacc
import concourse.tile as tile
from concourse import mybir
from contextlib import ExitStack


def _single_act_table(orig):
    """Route every exp/ln/copy/square/identity activation to the one ACT
    table set that contains BOTH exp and ln.  The default chooser assigns
    exp->exp_and_others and ln->natural_log, which thrashes table reloads
    (~1.3us each) on every exp<->ln transition in the batchnorm second pass.
    Removing those funcs from every other set leaves the chooser exactly one
    legal assignment; set ids keep their act_info.json indices so walrus
    loads the right tables."""
    AF = mybir.ActivationFunctionType
    both = {AF.Exp, AF.Ln}
    shared = {AF.Exp, AF.Ln, AF.Copy, AF.Square, AF.Identity}
    home = None
    for name, funcs in orig.items():
        if both <= funcs:
            home = name
            break
    if home is None:
        return orig
    out = {}
    for name, funcs in orig.items():
        out[name] = funcs if name == home else (funcs - shared)
    return out


_orig_get_activation_tables = bacc.get_activation_tables


def _patched_get_activation_tables(arch):
    return _single_act_table(_orig_get_activation_tables(arch))


bacc.get_activation_tables = _patched_get_activation_tables

F16 = mybir.dt.float16
F32 = mybir.dt.float32
I8 = mybir.dt.int8
I16 = mybir.dt.int16
I32 = mybir.dt.int32


def make_cfg(N=60000, M=12, F0=92, FB=41, F=64, H=128, NC=2000, NCONV=3,
             EPS=1e-5, NCORES=8, TA=128):
    c = dict(N=N, M=M, F0=F0, FB=FB, F=F, H=H, NC=NC, NCONV=NCONV, EPS=EPS,
             NCORES=NCORES, TA=TA)
    assert N % NCORES == 0
    c["A_shard"] = N // NCORES
    c["ntile"] = (c["A_shard"] + TA - 1) // TA
    c["A_pad"] = c["ntile"] * TA
    c["E_tile"] = TA * M
    c["E_loc"] = c["ntile"] * c["E_tile"]
    assert (NCORES * c["A_pad"]) % 2 == 0
    c["R2"] = 1 + NCORES * c["A_pad"] // 2      # pair-table rows (zero row at 0)
    assert c["R2"] <= 32768, "pair table must stay int16-addressable"
    c["W16"] = c["E_tile"] // 16
    c["NCB"] = 512 * ((NC + 511) // 512) + 512  # crystal bounce rows
    c["G"] = 2 * F                              # gated channels
    return c


CFG = make_cfg()


# --------------------------------------------------------------------------
# program builder
# --------------------------------------------------------------------------

def build_program(c, debug=False, dbg_dump=False, stop=None):
    # stop: optional (layer, stage) tuple for bisection; stages within a layer:
    # 0=Y, 1=AG, 2=A, 3=AR1, 4=B, 5=AR2, 6=C; (L,0)=pool, (L,1)=head
    nc = bacc.Bacc("TRN2", target_bir_lowering=False, debug=debug,
                   num_devices=c["NCORES"])

    N, M, F0, FB, F, H, NC, L = (c["N"], c["M"], c["F0"], c["FB"], c["F"],
                                 c["H"], c["NC"], c["NCONV"])
    G, TA, NT, AP_, ET, EL = (c["G"], c["TA"], c["ntile"], c["A_pad"],
                              c["E_tile"], c["E_loc"])
    R2, W16, NCB, EPS = c["R2"], c["W16"], c["NCB"], c["EPS"]
    NPAIR = (NT + 1) // 2
    NCHUNK = ET // TA                           # 128-edge chunks per tile

    # ---------------- inputs ----------------
    afT = nc.dram_tensor("afT", [F0, AP_], F16, kind="ExternalInput")
    idxw = nc.dram_tensor("idxw", [128, NT, W16], I16, kind="ExternalInput")
    mparr = nc.dram_tensor("mparr", [128, NT, M], I8, kind="ExternalInput")
    nbrT = nc.dram_tensor("nbrT", [FB, EL], F16, kind="ExternalInput")
    oh_self = nc.dram_tensor("oh_self", [128, ET], F16, kind="ExternalInput")
    pone = nc.dram_tensor("pone", [NT, 128, 512], F16, kind="ExternalInput")
    scidx = nc.dram_tensor("scidx", [128, 4], I32, kind="ExternalInput")
    invcnt = nc.dram_tensor("invcnt", [NCB, 1], F32, kind="ExternalInput")
    w_emb = nc.dram_tensor("w_emb", [F0, F], F16, kind="ExternalInput")
    b_emb = nc.dram_tensor("b_emb", [F, 1], F32, kind="ExternalInput")
    w_self = nc.dram_tensor("w_self", [L, F, G], F16, kind="ExternalInput")
    w_nbr = nc.dram_tensor("w_nbr", [L, F, G], F16, kind="ExternalInput")
    w_b = nc.dram_tensor("w_b", [L, FB, G], F16, kind="ExternalInput")
    g1 = nc.dram_tensor("g1", [L, G, 1], F32, kind="ExternalInput")
    be1 = nc.dram_tensor("be1", [L, G, 1], F32, kind="ExternalInput")
    g2 = nc.dram_tensor("g2", [L, F, 1], F32, kind="ExternalInput")
    be2 = nc.dram_tensor("be2", [L, F, 1], F32, kind="ExternalInput")
    w_fc = nc.dram_tensor("w_fc", [F, H], F16, kind="ExternalInput")
    b_fc = nc.dram_tensor("b_fc", [H, 1], F32, kind="ExternalInput")
    w_out = nc.dram_tensor("w_out", [H, 1], F16, kind="ExternalInput")
    b_out = nc.dram_tensor("b_out", [1, 1], F32, kind="ExternalInput")
    ident = nc.dram_tensor("ident", [128, 128], F32, kind="ExternalInput")
    identh = nc.dram_tensor("identh", [128, 128], F16, kind="ExternalInput")

    out_t = nc.dram_tensor("out", [1, NCB], F32, kind="ExternalOutput")
    if dbg_dump:
        L_, F_, G_, AP2, EL_ = c["NCONV"], c["F"], c["G"], c["A_pad"], c["E_loc"]
        dbgx0 = nc.dram_tensor("dbgx0", [F_, AP2], F32, kind="ExternalOutput")
        dbgx = nc.dram_tensor("dbgx", [L_, F_, AP2], F32, kind="ExternalOutput")
        dbgsum = nc.dram_tensor("dbgsum", [L_, F_, AP2], F32, kind="ExternalOutput")
        dbgst1 = nc.dram_tensor("dbgst1", [L_, G_, 2], F32, kind="ExternalOutput")
        dbggat = nc.dram_tensor("dbggat", [L_, NT, 128, ET], F16,
                                kind="ExternalOutput")
        dbgpool = nc.dram_tensor("dbgpool", [NCB, F_], F32, kind="ExternalOutput")

    # ---------------- internal DRAM ----------------
    yb = nc.dram_tensor("yb", [AP_, G], F16)                        # AG input bounce
    tbl = nc.dram_tensor("tbl", [R2, 2 * G], F16, addr_space="Shared")
    scr = nc.dram_tensor("scr", [NT, 128, ET], F16)
    ar1_in = nc.dram_tensor("ar1_in", [G, 2], F32)
    ar1_out = nc.dram_tensor("ar1_out", [G, 2], F32, addr_space="Shared")
    ar2_in = nc.dram_tensor("ar2_in", [F, 2], F32)
    ar2_out = nc.dram_tensor("ar2_out", [F, 2], F32, addr_space="Shared")
    pool_in = nc.dram_tensor("pool_in", [NCB, F], F32)
    pool_out = nc.dram_tensor("pool_out", [NCB, F], F32, addr_space="Shared")

    rg = [list(range(c["NCORES"]))]
    AF = mybir.ActivationFunctionType
    OP = mybir.AluOpType
    if stop is None:
        stop = (L, 9)

    def live(key):
        return key <= stop

    with tile.TileContext(nc) as tc, ExitStack() as top:
        # persistent SBUF state
        x_cm = nc.alloc_sbuf_tensor("x_cm", [F, AP_], F32)
        summed = nc.alloc_sbuf_tensor("summed", [F, AP_], F16)
        ysr = nc.alloc_sbuf_tensor("ysr", [128, NT, G], F16)      # Y_self row-major
        idx_all = nc.alloc_sbuf_tensor("idx_all", [128, NT, W16], I16)
        mpar = nc.alloc_sbuf_tensor("mpar", [128, NT, M, 1], I8)

        const = top.enter_context(tc.tile_pool(name="const", bufs=1))
        stats = top.enter_context(tc.tile_pool(name="stats", bufs=1))

        # constants resident all kernel
        ohs_t = const.tile([128, ET], F16)
        nc.sync.dma_start(out=ohs_t[:], in_=oh_self[:, :])
        wemb_t = const.tile([F0, F], F16)
        nc.sync.dma_start(out=wemb_t[:], in_=w_emb[:, :])
        bemb_t = const.tile([F, 1], F32)
        nc.sync.dma_start(out=bemb_t[:], in_=b_emb[:, :])
        id_t = const.tile([128, 128], F32)
        nc.sync.dma_start(out=id_t[:], in_=ident[:, :])
        idh_t = const.tile([128, 128], F16)
        nc.sync.dma_start(out=idh_t[:], in_=identh[:, :])
        eps_t = const.tile([128, 1], F32)
        nc.vector.memset(eps_t[:], EPS)
        zrow = const.tile([1, 2 * G], F16)
        nc.vector.memset(zrow[:], 0.0)
        zt128 = const.tile([128, F], F32)
        nc.vector.memset(zt128[:], 0.0)

        # layer-invariant gather indices and parity masks
        nc.sync.dma_start(out=idx_all[:, :, :], in_=idxw[:, :, :])
        nc.sync.dma_start(out=mpar[:, :, :, 0], in_=mparr[:, :, :])

        wS = []
        wN = []
        wB = []
        for l in range(L):
            t1 = const.tile([F, G], F16, tag=f"wS{l}")
            nc.sync.dma_start(out=t1[:], in_=w_self[l, :, :])
            t2 = const.tile([F, G], F16, tag=f"wN{l}")
            nc.sync.dma_start(out=t2[:], in_=w_nbr[l, :, :])
            t3 = const.tile([FB, G], F16, tag=f"wB{l}")
            nc.sync.dma_start(out=t3[:], in_=w_b[l, :, :])
            wS.append(t1)
            wN.append(t2)
            wB.append(t3)

        # stats buffers
        st1_s = stats.tile([G, NT], F32, tag="st1s")
        st1_q = stats.tile([G, NT], F32, tag="st1q")
        st2_s = stats.tile([F, 2 * NPAIR], F32, tag="st2s")
        st2_q = stats.tile([F, 2 * NPAIR], F32, tag="st2q")

        # zero table guard row + summed pads
        nc.sync.dma_start(out=tbl[0:1, :], in_=zrow[:])
        nc.vector.memset(summed[:, :], 0.0)

        # ---------------- embedding: x = atom_fea @ W_emb + b_emb ----------
        with tc.tile_pool(name="emb", bufs=3) as embp, \
             tc.tile_pool(name="embps", bufs=2, space="PSUM") as embps:
            CH = 512
            for j in range(0, AP_, CH):
                w = min(CH, AP_ - j)
                rhs = embp.tile([F0, CH], F16, tag="embr")
                nc.sync.dma_start(out=rhs[:, :w], in_=afT[:, j:j + w])
                ps = embps.tile([F, CH], F32, tag="embp")
                nc.tensor.matmul(ps[:, :w], lhsT=wemb_t[:], rhs=rhs[:, :w],
                                 start=True, stop=True)
                nc.scalar.activation(out=x_cm[:, j:j + w], in_=ps[:, :w],
                                     func=AF.Identity, bias=bemb_t[:], scale=1.0)
        if dbg_dump:
            nc.sync.dma_start(out=dbgx0[:, :], in_=x_cm[:, :])

        # ---------------- conv layers ----------------
        for l in range(L):
            if not live((l, 0)):
                break
            # ---- phase Y: Y_self (SBUF) / Y_nbr (-> bounce -> AllGather) ----
            with tc.tile_pool(name="yph", bufs=3) as yp, \
                 tc.tile_pool(name="yps", bufs=2, space="PSUM") as yps:
                lastreal = c["A_shard"] - (NT - 1) * TA
                for t in range(NT):
                    xa = yp.tile([F, TA], F16, tag="xa")
                    nc.scalar.activation(out=xa[:], in_=x_cm[:, t * TA:(t + 1) * TA],
                                         func=AF.Copy)
                    psS = yps.tile([TA, G], F32, tag="psS")
                    nc.tensor.matmul(psS[:], lhsT=xa[:], rhs=wS[l][:],
                                     start=True, stop=True)
                    # pad atoms of the last tile must contribute exactly zero
                    # through the self one-hot matmul
                    nreal = TA if t < NT - 1 else lastreal
                    if nreal < TA:
                        nc.vector.memset(ysr[:, t, :], 0.0)
                    nc.scalar.activation(out=ysr[0:nreal, t, :],
                                         in_=psS[0:nreal, :], func=AF.Copy)
                    psN = yps.tile([TA, G], F32, tag="psN")
                    nc.tensor.matmul(psN[:], lhsT=xa[:], rhs=wN[l][:],
                                     start=True, stop=True)
                    yn = yp.tile([TA, G], F16, tag="yn")
                    nc.scalar.activation(out=yn[:], in_=psN[:], func=AF.Copy)
                    nc.sync.dma_start(out=yb[t * TA:(t + 1) * TA, :], in_=yn[:])

            if not live((l, 1)):
                break
            tc.strict_bb_all_engine_barrier()
            nc.gpsimd.collective_compute(
                "AllGather", OP.bypass, replica_groups=rg,
                ins=[yb[:, :]], outs=[tbl[1:R2, :]])
            tc.strict_bb_all_engine_barrier()

            if not live((l, 2)):
                break
            # ---- pass A: edges -> gated scratch + stats1 ----
            with tc.tile_pool(name="pa", bufs=4) as pa, \
                 tc.tile_pool(name="paps", bufs=2, space="PSUM") as paps:
                for t in range(NT):
                    nbt = pa.tile([FB, ET], F16, tag="nbt")
                    nc.sync.dma_start(out=nbt[:], in_=nbrT[:, t * ET:(t + 1) * ET])

                    ge = pa.tile([128, NCHUNK, 2 * G], F16, tag="ge")
                    nc.gpsimd.dma_gather(ge[:], tbl[:, :], idx_all[:, t, :],
                                         ET, ET, 2 * G, single_packet=False)
                    # parity select: overwrite even-atom slab with odd-atom
                    # slab wherever the edge's target index is odd
                    nc.vector.copy_predicated(
                        out=ge[:, :, 0:G],
                        mask=mpar[:, t, :, :].broadcast_to([128, NCHUNK, G]),
                        data=ge[:, :, G:2 * G])

                    # every 128-col region: chunk transpose (start) -> wB ->
                    # one-hot self term (stop)
                    ps = paps.tile([G, ET], F32, tag="aps")
                    for i in range(NCHUNK):
                        cs = slice(i * 128, (i + 1) * 128)
                        nc.tensor.matmul(ps[:, cs], lhsT=ge[:, i, 0:G],
                                         rhs=idh_t[:], start=(i % 4 == 0),
                                         stop=False)
                    for s in range(ET // 512):
                        sl = slice(s * 512, (s + 1) * 512)
                        nc.tensor.matmul(ps[:, sl], lhsT=wB[l][:], rhs=nbt[:, sl],
                                         start=False, stop=False)
                        nc.tensor.matmul(ps[:, sl], lhsT=ysr[:, t, :],
                                         rhs=ohs_t[:, sl], start=False, stop=True)

                    gat = pa.tile([128, ET], F16, tag="gat")
                    nc.scalar.activation(out=gat[:], in_=ps[:], func=AF.Copy,
                                         accum_out=st1_s[:, t:t + 1])
                    sqd = pa.tile([128, ET], F16, tag="sqd")
                    nc.scalar.activation(out=sqd[:], in_=ps[:], func=AF.Square,
                                         accum_out=st1_q[:, t:t + 1])
                    nc.sync.dma_start(out=scr[t, :, :], in_=gat[:])

            if not live((l, 3)):
                break
            # ---- stats1 reduce + AllReduce + affine ----
            with tc.tile_pool(name="s1", bufs=1) as s1p:
                pack1 = s1p.tile([G, 2], F32, tag="pack1")
                nc.vector.tensor_reduce(out=pack1[:, 0:1], in_=st1_s[:],
                                        axis=mybir.AxisListType.X, op=OP.add)
                nc.vector.tensor_reduce(out=pack1[:, 1:2], in_=st1_q[:],
                                        axis=mybir.AxisListType.X, op=OP.add)
                nc.sync.dma_start(out=ar1_in[:, :], in_=pack1[:])
                tc.strict_bb_all_engine_barrier()
                nc.gpsimd.collective_compute(
                    "AllReduce", OP.add, replica_groups=rg,
                    ins=[ar1_in[:, :]], outs=[ar1_out[:, :]])
                tc.strict_bb_all_engine_barrier()

                red1 = s1p.tile([G, 2], F32, tag="red1")
                nc.sync.dma_start(out=red1[:], in_=ar1_out[:, :])
                if dbg_dump:
                    nc.sync.dma_start(out=dbgst1[l, :, :], in_=red1[:])
                    for t in range(NT):
                        nc.gpsimd.dma_start(out=dbggat[l, t, :, :],
                                            in_=scr[t, :, :])
                g1_t = s1p.tile([G, 1], F32, tag="g1t")
                nc.sync.dma_start(out=g1_t[:], in_=g1[l, :, :])
                be1_t = s1p.tile([G, 1], F32, tag="be1t")
                nc.sync.dma_start(out=be1_t[:], in_=be1[l, :, :])

                # rsqrt(var+eps) = exp(-0.5*ln(var+eps)) + one Newton step
                # (no sqrt table needed; Ln/Exp share one ACT table set)
                invE = 1.0 / float(N * M)
                mmt = s1p.tile([G, 8], F32, tag="mmt")
                mcol = mmt[:, 0:1]
                nc.vector.tensor_scalar(out=mcol, in0=red1[:, 0:1], scalar1=invE,
                                        scalar2=None, op0=OP.mult)
                ex2 = mmt[:, 1:2]
                nc.vector.tensor_scalar(out=ex2, in0=red1[:, 1:2], scalar1=invE,
                                        scalar2=None, op0=OP.mult)
                msq = mmt[:, 2:3]
                nc.vector.tensor_tensor(out=msq, in0=mcol, in1=mcol, op=OP.mult)
                var = mmt[:, 3:4]
                nc.vector.tensor_tensor(out=var, in0=ex2, in1=msq, op=OP.subtract)
                lv = mmt[:, 4:5]
                nc.scalar.activation(out=lv, in_=var, func=AF.Ln,
                                     bias=eps_t[0:G, :], scale=1.0)
                r0 = mmt[:, 5:6]
                nc.scalar.activation(out=r0, in_=lv, func=AF.Exp, scale=-0.5)
                # one Newton step: r1 = r0*(1.5 - 0.5*(var+eps)*r0^2)
                vpe = mmt[:, 6:7]
                nc.vector.tensor_scalar(out=vpe, in0=var, scalar1=eps_t[0:G, :],
                                        scalar2=0.5, op0=OP.add, op1=OP.mult)
                r0q = mmt[:, 7:8]
                nc.vector.tensor_tensor(out=r0q, in0=r0, in1=r0, op=OP.mult)
                nc.vector.tensor_tensor(out=r0q, in0=r0q, in1=vpe, op=OP.mult)
                nc.vector.tensor_scalar(out=r0q, in0=r0q, scalar1=-1.0,
                                        scalar2=1.5, op0=OP.mult, op1=OP.add)
                r1 = mmt[:, 6:7]
                nc.vector.tensor_tensor(out=r1, in0=r0, in1=r0q, op=OP.mult)

                s1c_ = s1p.tile([G, 1], F32, tag="s1c")
                nc.vector.tensor_tensor(out=s1c_[:], in0=g1_t[:], in1=r1,
                                        op=OP.mult)
                t1c_ = s1p.tile([G, 1], F32, tag="t1c")
                nc.vector.tensor_tensor(out=t1c_[:], in0=mcol, in1=s1c_[:],
                                        op=OP.mult)
                nc.vector.scalar_tensor_tensor(out=t1c_[:], in0=t1c_[:],
                                               scalar=-1.0, in1=be1_t[:],
                                               op0=OP.mult, op1=OP.add)
                # negated F-half affine (sigmoid via 1/(1+exp(-a)))
                s1n = s1p.tile([G, 1], F32, tag="s1n")
                nc.vector.tensor_scalar(out=s1n[:], in0=s1c_[:], scalar1=-1.0,
                                        scalar2=None, op0=OP.mult)
                t1n = s1p.tile([G, 1], F32, tag="t1n")
                nc.vector.tensor_scalar(out=t1n[:], in0=t1c_[:], scalar1=-1.0,
                                        scalar2=None, op0=OP.mult)
                # replicated (packed-pair) scale/bias
                sF = s1p.tile([128, 1], F32, tag="sF")
                tF = s1p.tile([128, 1], F32, tag="tF")
                sC = s1p.tile([128, 1], F32, tag="sC")
                tC = s1p.tile([128, 1], F32, tag="tC")
                for half in range(2):
                    hp = slice(half * F, half * F + F)
                    nc.sync.dma_start(out=sF[hp, :], in_=s1n[0:F, :])
                    nc.sync.dma_start(out=tF[hp, :], in_=t1n[0:F, :])
                    nc.sync.dma_start(out=sC[hp, :], in_=s1c_[F:G, :])
                    nc.sync.dma_start(out=tC[hp, :], in_=t1c_[F:G, :])

                if not live((l, 4)):
                    break
                # ---- pass B: sigmoid*softplus, neighbor-sum, stats2 ----
                tc.strict_bb_all_engine_barrier()
                with tc.tile_pool(name="pb", bufs=3) as bp:
                    for p in range(NPAIR):
                        t0, t1_ = 2 * p, min(2 * p + 1, NT - 1)
                        single = (2 * p + 1 > NT - 1)
                        pp = 128 if not single else F
                        zf = bp.tile([128, ET], F16, tag="zf")
                        zc = bp.tile([128, ET], F16, tag="zc")
                        nc.sync.dma_start(out=zf[0:F, :], in_=scr[t0, 0:F, :])
                        nc.sync.dma_start(out=zc[0:F, :], in_=scr[t0, F:G, :])
                        if not single:
                            nc.sync.dma_start(out=zf[F:G, :], in_=scr[t1_, 0:F, :])
                            nc.sync.dma_start(out=zc[F:G, :], in_=scr[t1_, F:G, :])
                        # ef = 1 + exp(-(sF*zf+tF)) in f32 (no inf/denorm, so
                        # the bit-trick reciprocal seed is safe)
                        ef = bp.tile([128, ET], F32, tag="ef")
                        nc.scalar.activation(out=ef[0:pp, :], in_=zf[0:pp, :],
                                             func=AF.Exp,
                                             bias=tF[0:pp, :], scale=sF[0:pp, :])
                        ec = bp.tile([128, ET], F32, tag="ec")
                        nc.scalar.activation(out=ec[0:pp, :], in_=zc[0:pp, :],
                                             func=AF.Exp,
                                             bias=tC[0:pp, :], scale=sC[0:pp, :])
                        sp = bp.tile([128, ET], F16, tag="sp")
                        nc.scalar.activation(out=sp[0:pp, :], in_=ec[0:pp, :],
                                             func=AF.Ln, bias=1.0, scale=1.0)
                        nc.vector.tensor_scalar(out=ef[0:pp, :], in0=ef[0:pp, :],
                                                scalar1=1.0, scalar2=None,
                                                op0=OP.add)
                        sg = bp.tile([128, ET], F32, tag="sg")
                        nc.vector.reciprocal_approx_fast(out=sg[0:pp, :],
                                                         in_=ef[0:pp, :])
                        nc.vector.tensor_tensor(out=sp[0:pp, :], in0=sp[0:pp, :],
                                                in1=sg[0:pp, :], op=OP.mult)
                        halves = 1 if single else 2
                        for hh in range(halves):
                            tt = t0 if hh == 0 else t1_
                            hs = slice(hh * F, hh * F + F)
                            red = bp.tile([F, TA], F32, tag="red")
                            nc.vector.tensor_reduce(
                                out=red[:],
                                in_=sp[hs, :].rearrange("p (a m) -> p a m", m=M),
                                axis=mybir.AxisListType.X, op=OP.add)
                            nc.scalar.activation(
                                out=summed[:, tt * TA:(tt + 1) * TA],
                                in_=red[:], func=AF.Copy)
                            # stats over real atoms only
                            nreal = min(c["A_shard"] - tt * TA, TA)
                            col = 2 * p + hh
                            nc.vector.tensor_reduce(
                                out=st2_s[:, col:col + 1], in_=red[:, 0:nreal],
                                axis=mybir.AxisListType.X, op=OP.add)
                            sqt = bp.tile([F, TA], F32, tag="sqt")
                            nc.scalar.activation(
                                out=sqt[:, 0:nreal], in_=red[:, 0:nreal],
                                func=AF.Square,
                                accum_out=st2_q[:, col:col + 1])

                if not live((l, 5)):
                    break
                # ---- stats2 AllReduce + affine2 + pass C ----
                ncols = NT
                pack2 = s1p.tile([F, 2], F32, tag="pack2")
                nc.vector.tensor_reduce(out=pack2[:, 0:1],
                                        in_=st2_s[:, 0:ncols],
                                        axis=mybir.AxisListType.X, op=OP.add)
                nc.vector.tensor_reduce(out=pack2[:, 1:2],
                                        in_=st2_q[:, 0:ncols],
                                        axis=mybir.AxisListType.X, op=OP.add)
                nc.sync.dma_start(out=ar2_in[:, :], in_=pack2[:])
                tc.strict_bb_all_engine_barrier()
                nc.gpsimd.collective_compute(
                    "AllReduce", OP.add, replica_groups=rg,
                    ins=[ar2_in[:, :]], outs=[ar2_out[:, :]])
                tc.strict_bb_all_engine_barrier()

                red2 = s1p.tile([F, 2], F32, tag="red2")
                nc.sync.dma_start(out=red2[:], in_=ar2_out[:, :])
                g2_t = s1p.tile([F, 1], F32, tag="g2t")
                nc.sync.dma_start(out=g2_t[:], in_=g2[l, :, :])
                be2_t = s1p.tile([F, 1], F32, tag="be2t")
                nc.sync.dma_start(out=be2_t[:], in_=be2[l, :, :])

                invN = 1.0 / float(N)
                mt2 = s1p.tile([F, 8], F32, tag="mt2")
                m2c = mt2[:, 0:1]
                nc.vector.tensor_scalar(out=m2c, in0=red2[:, 0:1], scalar1=invN,
                                        scalar2=None, op0=OP.mult)
                e2c = mt2[:, 1:2]
                nc.vector.tensor_scalar(out=e2c, in0=red2[:, 1:2], scalar1=invN,
                                        scalar2=None, op0=OP.mult)
                ms2 = mt2[:, 2:3]
                nc.vector.tensor_tensor(out=ms2, in0=m2c, in1=m2c, op=OP.mult)
                v2 = mt2[:, 3:4]
                nc.vector.tensor_tensor(out=v2, in0=e2c, in1=ms2, op=OP.subtract)
                lv2 = mt2[:, 4:5]
                nc.scalar.activation(out=lv2, in_=v2, func=AF.Ln,
                                     bias=eps_t[0:F, :], scale=1.0)
                r02 = mt2[:, 5:6]
                nc.scalar.activation(out=r02, in_=lv2, func=AF.Exp, scale=-0.5)
                vpe2 = mt2[:, 6:7]
                nc.vector.tensor_scalar(out=vpe2, in0=v2, scalar1=eps_t[0:F, :],
                                        scalar2=0.5, op0=OP.add, op1=OP.mult)
                r0q2 = mt2[:, 7:8]
                nc.vector.tensor_tensor(out=r0q2, in0=r02, in1=r02, op=OP.mult)
                nc.vector.tensor_tensor(out=r0q2, in0=r0q2, in1=vpe2, op=OP.mult)
                nc.vector.tensor_scalar(out=r0q2, in0=r0q2, scalar1=-1.0,
                                        scalar2=1.5, op0=OP.mult, op1=OP.add)
                r12 = mt2[:, 6:7]
                nc.vector.tensor_tensor(out=r12, in0=r02, in1=r0q2, op=OP.mult)
                s2c = s1p.tile([F, 1], F32, tag="s2c")
                nc.vector.tensor_tensor(out=s2c[:], in0=g2_t[:], in1=r12,
                                        op=OP.mult)
                t2c = s1p.tile([F, 1], F32, tag="t2c")
                nc.vector.tensor_tensor(out=t2c[:], in0=m2c, in1=s2c[:],
                                        op=OP.mult)
                nc.vector.scalar_tensor_tensor(out=t2c[:], in0=t2c[:],
                                               scalar=-1.0, in1=be2_t[:],
                                               op0=OP.mult, op1=OP.add)

                if not live((l, 6)):
                    break
                # pass C: x = softplus(x + s2*summed + t2) via ln(1+exp)
                with tc.tile_pool(name="pc", bufs=2) as pcp:
                    CW = 2048
                    for j in range(0, AP_, CW):
                        w = min(CW, AP_ - j)
                        pre = pcp.tile([F, CW], F32, tag="pre")
                        nc.vector.scalar_tensor_tensor(
                            out=pre[:, :w], in0=summed[:, j:j + w],
                            scalar=s2c[:, 0:1], in1=x_cm[:, j:j + w],
                            op0=OP.mult, op1=OP.add)
                        pex = pcp.tile([F, CW], F32, tag="pex")
                        nc.scalar.activation(out=pex[:, :w], in_=pre[:, :w],
                                             func=AF.Exp, bias=t2c[:], scale=1.0)
                        nc.scalar.activation(out=x_cm[:, j:j + w], in_=pex[:, :w],
                                             func=AF.Ln, bias=1.0, scale=1.0)
                if dbg_dump:
                    nc.gpsimd.dma_start(out=dbgsum[l, :, :], in_=summed[:, :])
                    nc.sync.dma_start(out=dbgx[l, :, :], in_=x_cm[:, :])

        for _ in range(1 if live((L, 0)) else 0):
            # ---------------- crystal pooling ----------------
            with tc.tile_pool(name="pool", bufs=4) as pp_, \
                 tc.tile_pool(name="poolacc", bufs=1, space="PSUM") as pacc, \
                 tc.tile_pool(name="poolps", bufs=2, space="PSUM") as pps:
                acc = pacc.tile([F, 512], F32, tag="pacc")
                for t in range(NT):
                    xps = pps.tile([TA, F], F32, tag="xps")
                    nc.tensor.transpose(out=xps[:], in_=x_cm[:, t * TA:(t + 1) * TA],
                                        identity=id_t[0:F, 0:F])
                    xrm = pp_.tile([TA, F], F16, tag="xrm")
                    nc.scalar.activation(out=xrm[:], in_=xps[:], func=AF.Copy)
                    oht = pp_.tile([128, 512], F16, tag="oht")
                    nc.sync.dma_start(out=oht[:], in_=pone[t, :, :])
                    nc.tensor.matmul(acc[:], lhsT=xrm[:], rhs=oht[:],
                                     start=(t == 0), stop=(t == NT - 1))
                # zero pool_in, then scatter our window rows
                for r_ in range(NCB // 128):
                    nc.sync.dma_start(out=pool_in[128 * r_:128 * (r_ + 1), :],
                                      in_=zt128[:, 0:F])
                sci = pp_.tile([128, 4], I32, tag="sci")
                nc.sync.dma_start(out=sci[:], in_=scidx[:, :])
                tc.strict_bb_all_engine_barrier()
                accS = pp_.tile([F, 512], F32, tag="accS")
                nc.vector.tensor_copy(out=accS[:], in_=acc[:])
                accs = []
                for ci in range(4):
                    # [F, 128] crystal block -> row-major [128, F]
                    tps = pps.tile([128, F], F32, tag="tps")
                    nc.tensor.transpose(out=tps[:],
                                        in_=accS[:, ci * 128:(ci + 1) * 128],
                                        identity=id_t[0:F, 0:F])
                    asb = pp_.tile([128, F], F32, tag=f"asb{ci}")
                    nc.vector.tensor_copy(out=asb[:], in_=tps[:])
                    accs.append(asb)
                for ci in range(4):
                    nc.gpsimd.indirect_dma_start(
                        out=pool_in[:, :],
                        out_offset=bass.IndirectOffsetOnAxis(ap=sci[:, ci:ci + 1], axis=0),
                        in_=accs[ci][:], in_offset=None)
                tc.strict_bb_all_engine_barrier()
                nc.gpsimd.collective_compute(
                    "AllReduce", OP.add, replica_groups=rg,
                    ins=[pool_in[:, :]], outs=[pool_out[:, :]])
                tc.strict_bb_all_engine_barrier()

            if dbg_dump:
                nc.sync.dma_start(out=dbgpool[:, :], in_=pool_out[:, :])


        for _ in range(1 if live((L, 1)) else 0):
            # ---------------- head ----------------
            with tc.tile_pool(name="head", bufs=3) as hp, \
                 tc.tile_pool(name="headps", bufs=2, space="PSUM") as hps:
                crys = nc.alloc_sbuf_tensor("crys", [F, NCB], F16)
                wfc_t = hp.tile([F, H], F16, tag="wfc")
                nc.sync.dma_start(out=wfc_t[:], in_=w_fc[:, :])
                bfc_t = hp.tile([H, 1], F32, tag="bfc")
                nc.sync.dma_start(out=bfc_t[:], in_=b_fc[:, :])
                wout_t = hp.tile([H, 1], F16, tag="wout")
                nc.sync.dma_start(out=wout_t[:], in_=w_out[:, :])
                bout_t = hp.tile([1, 1], F32, tag="bout")
                nc.sync.dma_start(out=bout_t[:], in_=b_out[:, :])

                for r_ in range(NCB // 128):
                    pt = hp.tile([128, F], F32, tag="pt")
                    nc.sync.dma_start(out=pt[:], in_=pool_out[128 * r_:128 * (r_ + 1), :])
                    ic = hp.tile([128, 1], F32, tag="ic")
                    nc.sync.dma_start(out=ic[:], in_=invcnt[128 * r_:128 * (r_ + 1), :])
                    nc.vector.tensor_scalar(out=pt[:], in0=pt[:], scalar1=ic[:, 0:1],
                                            scalar2=None, op0=OP.mult)
                    pex2 = hp.tile([128, F], F32, tag="pex2")
                    nc.scalar.activation(out=pex2[:], in_=pt[:], func=AF.Exp)
                    spt = hp.tile([128, F], F32, tag="spt")
                    nc.scalar.activation(out=spt[:], in_=pex2[:], func=AF.Ln,
                                         bias=1.0, scale=1.0)
                    tps = hps.tile([F, 128], F32, tag="tps")
                    nc.tensor.transpose(out=tps[:], in_=spt[:], identity=id_t[:, :])
                    nc.scalar.activation(out=crys[:, 128 * r_:128 * (r_ + 1)],
                                         in_=tps[:], func=AF.Copy)

                hc = nc.alloc_sbuf_tensor("hc", [H, NCB], F16)
                for j in range(0, NCB, 512):
                    psh = hps.tile([H, 512], F32, tag="psh")
                    nc.tensor.matmul(psh[:], lhsT=wfc_t[:], rhs=crys[:, j:j + 512],
                                     start=True, stop=True)
                    hex_ = hp.tile([H, 512], F32, tag="hex")
                    nc.scalar.activation(out=hex_[:], in_=psh[:],
                                         func=AF.Exp, bias=bfc_t[:], scale=1.0)
                    nc.scalar.activation(out=hc[:, j:j + 512], in_=hex_[:],
                                         func=AF.Ln, bias=1.0, scale=1.0)
                ofin = hp.tile([1, NCB], F32, tag="ofin")
                for j in range(0, NCB, 512):
                    pso = hps.tile([1, 512], F32, tag="pso")
                    nc.tensor.matmul(pso[:], lhsT=wout_t[:], rhs=hc[:, j:j + 512],
                                     start=True, stop=True)
                    nc.scalar.activation(out=ofin[:, j:j + 512], in_=pso[:],
                                         func=AF.Identity, bias=bout_t[:], scale=1.0)
                nc.sync.dma_start(out=out_t[:, :], in_=ofin[:])

    nc.compile()
    return nc


# --------------------------------------------------------------------------
# host-side input preparation
# --------------------------------------------------------------------------

def prepare_inputs(c, atom_fea, nbr_fea, nbr_fea_idx, crystal_atom_idx,
                   W_emb, b_emb, W_full, b_full, g1, be1, g2, be2,
                   W_fc, b_fc, W_out, b_out):
    N, M, F0, FB, F, H, NC, L = (c["N"], c["M"], c["F0"], c["FB"], c["F"],
                                 c["H"], c["NC"], c["NCONV"])
    G, TA, NT, AP_, ET, EL = (c["G"], c["TA"], c["ntile"], c["A_pad"],
                              c["E_tile"], c["E_loc"])
    W16, NCB, K = c["W16"], c["NCB"], c["NCORES"]
    AS = c["A_shard"]

    atom_fea = np.asarray(atom_fea, np.float32)
    nbr_fea = np.asarray(nbr_fea, np.float32)
    nbr_fea_idx = np.asarray(nbr_fea_idx, np.int64)
    crystal_atom_idx = np.asarray(crystal_atom_idx, np.int64)

    # shared (replicated) tensors
    oh = np.zeros((128, ET), np.float16)
    for j in range(ET):
        oh[j // M, j] = 1.0
    shared = {
        "oh_self": oh,
        "w_emb": np.asarray(W_emb, np.float16),
        "b_emb": np.asarray(b_emb, np.float32).reshape(F, 1),
        "w_self": np.asarray(W_full[:, :F, :], np.float16),
        "w_nbr": np.asarray(W_full[:, F:2 * F, :], np.float16),
        "w_b": np.asarray(W_full[:, 2 * F:, :], np.float16),
        "g1": np.asarray(g1, np.float32).reshape(L, G, 1),
        "be1": np.asarray(be1, np.float32).reshape(L, G, 1),
        "g2": np.asarray(g2, np.float32).reshape(L, F, 1),
        "be2": np.asarray(be2, np.float32).reshape(L, F, 1),
        "w_fc": np.asarray(W_fc, np.float16),
        "b_fc": np.asarray(b_fc, np.float32).reshape(H, 1),
        "w_out": np.asarray(W_out, np.float16),
        "b_out": np.asarray(b_out, np.float32).reshape(1, 1),
        "ident": np.eye(128, dtype=np.float32),
        "identh": np.eye(128, dtype=np.float16),
    }
    # crystal counts (global, from index data only)
    cnt = np.bincount(crystal_atom_idx, minlength=NC).astype(np.float32)
    icnt = np.zeros((NCB, 1), np.float32)
    icnt[:NC, 0] = 1.0 / np.maximum(cnt, 1.0)
    shared["invcnt"] = icnt

    # b_full is mathematically irrelevant (cancelled by training-mode BN)

    in_maps = []
    for k in range(K):
        a0 = k * AS
        af = np.zeros((F0, AP_), np.float16)
        af[:, :AS] = atom_fea[a0:a0 + AS].T
        # edge ordering: e = a*M + m within each tile of TA atoms
        gi = np.zeros((NT * TA, M), np.int64)
        gi_raw = nbr_fea_idx[a0:a0 + AS]
        gi[:AS] = gi_raw
        valid = np.zeros((NT * TA, M), bool)
        valid[:AS] = True
        jj = (gi // AS) * AP_ + (gi % AS)          # padded-global atom index
        rows = np.where(valid, 1 + jj // 2, 0).astype(np.int64)
        par = np.where(valid, jj & 1, 0).astype(np.int64)
        idxw = np.zeros((128, NT, W16), np.int16)
        mparr = np.zeros((128, NT, M), np.int8)
        j = np.arange(ET)
        for t in range(NT):
            fl = rows[t * TA:(t + 1) * TA].reshape(ET)
            wrap = np.zeros((16, W16), np.int16)
            wrap[j % 16, j // 16] = fl
            idxw[:, t, :] = np.tile(wrap, (8, 1))
            # ge[p, i, :] holds edge i*128+p -> mask[p, i]
            mparr[:, t, :] = par[t * TA:(t + 1) * TA].reshape(ET)[
                (np.arange(M)[None, :] * 128 + np.arange(128)[:, None])]
        nb = np.zeros((FB, EL), np.float16)
        nb_l = nbr_fea[a0:a0 + AS].reshape(AS * M, FB)
        src = np.zeros((NT * TA * M, FB), np.float32)
        src[:AS * M] = nb_l
        # src is already in (a, m) order; tiles are contiguous runs of ET
        nb[:, :] = src.T.astype(np.float16)

        cry = np.zeros(NT * TA, np.int64)
        cry[:AS] = crystal_atom_idx[a0:a0 + AS]
        cb = int(crystal_atom_idx[a0:a0 + AS].min())
        cmax = int(crystal_atom_idx[a0:a0 + AS].max())
        assert cmax - cb < 512, f"crystal window too wide: {cmax - cb}"
        # pool one-hot: [atom (partition), crystal-window col]
        pone = np.zeros((NT, TA, 512), np.float16)
        for t in range(NT):
            for a in range(TA):
                ga = t * TA + a
                if ga >= AS:
                    continue
                pone[t, a, int(cry[ga]) - cb] = 1.0
        scidx = (cb + np.arange(512, dtype=np.int32)).reshape(4, 128).T.copy()
        assert cb + 512 <= NCB

        in_maps.append(dict(shared,
                            afT=af, idxw=idxw, mparr=mparr, nbrT=nb, pone=pone,
                            scidx=scidx))
    return in_maps


# --------------------------------------------------------------------------
# public entry point
# --------------------------------------------------------------------------

_PROG_CACHE = {}


def _get_program(c):
    key = tuple(sorted((k, v) for k, v in c.items()))
    if key not in _PROG_CACHE:
        _PROG_CACHE[key] = build_program(c)
    return _PROG_CACHE[key]


def kernel(atom_fea, nbr_fea, nbr_fea_idx, crystal_atom_idx, W_emb, b_emb,
           W_full, b_full, g1, be1, g2, be2, W_fc, b_fc, W_out, b_out,
           _trace=False):
    from concourse import bass_utils
    c = CFG
    nc = _get_program(c)
    in_maps = prepare_inputs(c, atom_fea, nbr_fea, nbr_fea_idx,
                             crystal_atom_idx, W_emb, b_emb, W_full, b_full,
                             g1, be1, g2, be2, W_fc, b_fc, W_out, b_out)
    res = bass_utils.run_bass_kernel_spmd(
        nc, in_maps, core_ids=list(range(c["NCORES"])), trace=_trace)
    out = np.asarray(res.results[0]["out"], np.float32)
    ret = out[0, :c["NC"]].reshape(c["NC"], 1)
    if _trace:
        return ret, res
    return ret
